# revision 41
# baseline (speedup 1.0000x reference)
"""AttentiveItemToVec Trainium2 kernel (batched bf16, host-folded weights).

Full-input contract: kernel(**inputs) takes the unsharded numpy inputs and
returns the full [512, 101, 128] float32 output. Internally shards the batch
across 8 NeuronCores (64 batches each), runs a Bass/Tile kernel per core via
run_bass_kernel_spmd, and concatenates the per-core outputs.

Host prep folds weight-space linear maps (data-independent):
  tw = t_emb @ At_w^T + At_b  [V, 60] -> the gather IS the target projection
  W  = (R_w Bc_w)^T                   -> R folded into Bc (one output matmul)
  rbeff = R_b + R_w @ Bc_b            -> folded into the bu rows on device

Device (per core, 64 batches):
- Gathers use indirect DMA at its max of 128 rows/instruction (this
  runtime generates one descriptor per partition): 51 tight tw blocks +
  25 tight raw-u blocks = 76 instructions saturating the GpSimd queue;
  everything else is sized to hide under it.
- PE transposes flip gathered blocks feature-major (8 blocks per PSUM
  bank, one DVE 2x copy per bank). Context projection cp = uT^T Ac^T runs
  in 512-col chunks, bias folded into the PSUM->SBUF copy. bu = u W^T runs
  per batch (windowed lhsT keeps outputs at partition 0) with rbeff added
  during the grouped PSUM->SBUF copy.
- Norms: per-batch matmul-with-ones into shared PSUM columns, then 4
  batched Ln/Exp activations for 1/|x| = exp(-0.5*ln(x^2+eps)).
- Per-batch softmax chain: dot matmul -> DVE row scale (ntinv) -> PE
  transpose -> Act exp (scale=ncinv, bias=pad mask) -> cs matmul into a
  shared column. Group reciprocal per 16 batches; output stage (o matmul,
  then a pure rsinv scale split across DVE/Act) runs one group behind,
  flushed to HBM in 8-batch DMAs.
"""

import numpy as np
from contextlib import ExitStack

# Problem constants (hardcoded per contract).
V, E, D = 100000, 128, 60
B, J, M, P = 512, 101, 50, 5120
NCORES = 8
BLOC = B // NCORES          # 64 batches per core
NTW = BLOC * J              # 6464 target rows per core
NU = BLOC * M               # 3200 context rows per core
NTWB = (NTW + 127) // 128   # 51 tight tw blocks
NUB = NU // 128             # 25 tight u blocks
NEG = -1.0e30
EPS2 = 1e-12

_CACHE = {}

_ACT_TABLE = "natural_log_exp_and_others"


def _patched_tables(orig_fn):
    def fn(arch):
        tabs = orig_fn(arch)
        return {
            name: (s if name == _ACT_TABLE else type(s)())
            for name, s in tabs.items()
        }
    return fn


def _build_program():
    import os
    NOPATCH = os.environ.get("K_NOPATCH") == "1"
    import concourse.bass as bass
    import concourse.tile as tile
    import concourse.bacc as bacc_mod
    from concourse import bacc, mybir

    f32 = mybir.dt.float32
    bf16 = mybir.dt.bfloat16
    i32 = mybir.dt.int32

    nc = bacc.Bacc(
        "TRN2",
        target_bir_lowering=False,
        debug=False,
        enable_asserts=False,
    )

    twe = nc.dram_tensor("twe", [V, D], bf16, kind="ExternalInput").ap()
    cemb = nc.dram_tensor("cemb", [V, E], bf16, kind="ExternalInput").ap()
    acw = nc.dram_tensor("acw", [E, D], bf16, kind="ExternalInput").ap()
    wrb = nc.dram_tensor("wrb", [E, E], bf16, kind="ExternalInput").ap()
    acb = nc.dram_tensor("acb", [D, 1], f32, kind="ExternalInput").ap()
    rbeff = nc.dram_tensor("rbeff", [1, E], f32, kind="ExternalInput").ap()
    eye = nc.dram_tensor("eye", [128, 128], bf16, kind="ExternalInput").ap()
    offt = nc.dram_tensor("offt", [128, NTWB], i32, kind="ExternalInput").ap()
    offc = nc.dram_tensor("offc", [128, NUB], i32, kind="ExternalInput").ap()
    maskT = nc.dram_tensor("maskT", [M, BLOC], f32, kind="ExternalInput").ap()
    out = nc.dram_tensor("out", [BLOC, J, E], f32, kind="ExternalOutput").ap()

    AF = mybir.ActivationFunctionType
    ALU = mybir.AluOpType

    with tile.TileContext(nc) as tc, ExitStack() as ctx:
        const = ctx.enter_context(tc.tile_pool(name="const", bufs=1))
        big = ctx.enter_context(tc.tile_pool(name="big", bufs=1))
        outp = ctx.enter_context(tc.tile_pool(name="outp", bufs=3))
        dotp = ctx.enter_context(tc.tile_pool(name="dotp", bufs=8))

        # --- constants (gather offsets first so the SWDGE stream starts early)
        offc_t = const.tile([128, NUB], i32)
        nc.sync.dma_start(out=offc_t[:], in_=offc[:, :])
        offt_t = const.tile([128, NTWB], i32)
        nc.sync.dma_start(out=offt_t[:], in_=offt[:, :])
        eye_t = const.tile([128, 128], bf16)
        nc.sync.dma_start(out=eye_t[:], in_=eye[:, :])
        acw_t = const.tile([E, D], bf16)
        nc.sync.dma_start(out=acw_t[:], in_=acw[:, :])
        wrb_t = const.tile([E, E], bf16)
        nc.sync.dma_start(out=wrb_t[:], in_=wrb[:, :])
        acb_t = const.tile([D, 1], f32)
        nc.sync.dma_start(out=acb_t[:], in_=acb[:, :])
        rb4_t = const.tile([M, 4 * E], f32)
        rb4_bcast = bass.AP(tensor=rbeff.tensor, offset=0,
                            ap=[[0, M], [0, 4], [1, E]])
        rb4_dst = rb4_t[:]
        nc.sync.dma_start(
            out=bass.AP(tensor=rb4_dst.tensor, offset=rb4_dst.offset,
                        ap=[rb4_dst.ap[0], [E, 4], [1, E]]),
            in_=rb4_bcast,
        )
        maskT_t = const.tile([M, BLOC], f32)
        nc.sync.dma_start(out=maskT_t[:], in_=maskT[:, :])
        ones_t = const.tile([128, 1], bf16)
        nc.vector.memset(ones_t[:], 1.0)
        eps_t = const.tile([128, 1], f32)
        nc.vector.memset(eps_t[:], EPS2)

        # --- big SBUF arrays ---
        twg = big.tile([128, NTWB, D], bf16)     # gathered tw rows, tight
        ug = big.tile([128, NUB, E], bf16)       # gathered c_emb rows, tight
        tpT_all = big.tile([D, NTWB * 128], bf16)  # feature-major tp
        uT_all = big.tile([E, NU], bf16)         # feature-major u
        cpT_all = big.tile([D, NU], bf16)        # context projection (+Ac_b)
        tp2_all = big.tile([D, NTWB * 128], bf16)
        cp2_all = big.tile([D, NU], bf16)
        bu_all = big.tile([M, BLOC * E], bf16)   # u @ (R Bc)^T + rbeff
        attnT = big.tile([M, BLOC * J], bf16)
        ntinv = big.tile([128, BLOC], f32)
        ncinv = big.tile([M, BLOC], f32)
        lnt_t = big.tile([128, BLOC], f32)
        lnc_t = big.tile([M, BLOC], f32)
        rs_all = big.tile([128, BLOC], f32)

        # --- gathers: 128 rows per indirect DMA (one desc per partition).
        # u blocks first (they feed the early u-side pipeline), then tw.
        for c in range(NUB):
            nc.gpsimd.indirect_dma_start(
                out=ug[:, c, :], out_offset=None, in_=cemb[:, :],
                in_offset=bass.IndirectOffsetOnAxis(
                    ap=offc_t[:, c : c + 1], axis=0
                ),
            )
        for c in range(NTWB):
            nc.gpsimd.indirect_dma_start(
                out=twg[:, c, :], out_offset=None, in_=twe[:, :],
                in_offset=bass.IndirectOffsetOnAxis(
                    ap=offt_t[:, c : c + 1], axis=0
                ),
            )

        if True:
            ps_tr = ctx.enter_context(
                tc.tile_pool(name="ps_tr", bufs=1, space="PSUM"))
            ps_sm = ctx.enter_context(
                tc.tile_pool(name="ps_sm", bufs=1, space="PSUM"))

            # nt columns and cs columns share one PSUM bank
            sm_ps = ps_sm.tile([128, 128], f32, tag="sm", space="PSUM")
            nt_ps = sm_ps[:, :BLOC]
            cs_ps = sm_ps[:, BLOC : 2 * BLOC]

            # ---- u side: transposes, projection, bu, context norms ----
            with tc.tile_pool(name="ps_pj", bufs=2, space="PSUM") as ps_pj, \
                 tc.tile_pool(name="ps_nc", bufs=1, space="PSUM") as ps_nc:
                for g in range((NUB + 7) // 8):
                    nblk = min(8, NUB - 8 * g)
                    floor_ms = (3000 + 1038 * (8 * g + nblk)) * 1e-6
                    with tc.tile_wait_until(floor_ms):
                        tr = ps_tr.tile([128, 1024], bf16, tag="tr",
                                        space="PSUM")
                        for i in range(nblk):
                            nc.tensor.transpose(
                                out=tr[:, 128 * i : 128 * (i + 1)],
                                in_=ug[:, 8 * g + i, :],
                                identity=eye_t[:, :],
                            )
                        nc.vector.tensor_copy(
                            out=uT_all[:, 1024 * g : 1024 * g + 128 * nblk],
                            in_=tr[:, : 128 * nblk],
                        )
                for c in range((NU + 511) // 512):
                    s = 512 * c
                    w = min(512, NU - s)
                    pj = ps_pj.tile([128, 512], f32, tag="pj", space="PSUM")
                    nc.tensor.matmul(
                        out=pj[:D, :w], lhsT=acw_t[:], rhs=uT_all[:, s : s + w],
                        start=True, stop=True,
                    )
                    if c % 2 == 0:
                        nc.vector.tensor_scalar_add(
                            out=cpT_all[:, s : s + w], in0=pj[:D, :w],
                            scalar1=acb_t[:],
                        )
                    else:
                        nc.scalar.activation(
                            out=cpT_all[:, s : s + w], in_=pj[:D, :w],
                            func=AF.Identity, bias=acb_t[:], scale=1.0,
                        )
                    nc.vector.tensor_mul(
                        out=cp2_all[:, s : s + w],
                        in0=cpT_all[:, s : s + w],
                        in1=cpT_all[:, s : s + w],
                    )
                nc_ps = ps_nc.tile([M, BLOC], f32, tag="ncn", space="PSUM")
                for b in range(BLOC):
                    nc.tensor.matmul(
                        out=nc_ps[:, b : b + 1],
                        lhsT=cp2_all[:, M * b : M * (b + 1)],
                        rhs=ones_t[:D, :],
                        start=True, stop=True,
                    )
                nc.scalar.activation(
                    out=lnc_t[:], in_=nc_ps[:, :], func=AF.Ln, bias=eps_t[:M]
                )
                nc.scalar.activation(
                    out=ncinv[:], in_=lnc_t[:], func=AF.Exp, scale=-0.5
                )
                for c in range(BLOC // 4):
                    bu_ps = ps_pj.tile([128, 512], f32, tag="pj", space="PSUM")
                    for i in range(4):
                        b = 4 * c + i
                        nc.tensor.matmul(
                            out=bu_ps[:M, 128 * i : 128 * (i + 1)],
                            lhsT=uT_all[:, M * b : M * b + M],
                            rhs=wrb_t[:],
                            start=True, stop=True,
                        )
                    nc.vector.scalar_tensor_tensor(
                        out=bu_all[:, 512 * c : 512 * (c + 1)],
                        in0=bu_ps[:M, :], scalar=1.0,
                        in1=rb4_t[:],
                        op0=ALU.mult, op1=ALU.add,
                    )

            # ---- tw side + attention, interleaved with the gather stream ----
            # One PSUM bank per in-flight batch: dot at byte 0 (f32 [101,50]),
            # dotT at byte 1024 (bf16 [50,128] via bitcast). 4 banks = depth 4.
            ps_ch = ctx.enter_context(
                tc.tile_pool(name="ps_ch", bufs=4, space="PSUM"))
            ps_o = ctx.enter_context(
                tc.tile_pool(name="ps_o", bufs=2, space="PSUM"))
            GRP = 16
            pendB = []
            obuf = None

            def stageA(b):
                s = J * b
                ch = ps_ch.tile([128, 512], f32, tag="ch", space="PSUM")
                dot_ps = ch[:J, :M]
                nc.tensor.matmul(
                    out=dot_ps,
                    lhsT=tpT_all[:, s : s + J],
                    rhs=cpT_all[:, M * b : M * b + M],
                    start=True, stop=True,
                )
                dotn = dotp.tile([J, M], bf16, tag="dotn")
                nc.vector.tensor_scalar_mul(
                    dotn[:], dot_ps, ntinv[:J, b : b + 1]
                )
                dT_ps = ch[:M, 256:320].bitcast(bf16)
                nc.tensor.transpose(
                    out=dT_ps[:, :J], in_=dotn[:], identity=eye_t[:J, :J]
                )
                nc.scalar.activation(
                    out=attnT[:, s : s + J],
                    in_=dT_ps[:, :J],
                    func=AF.Exp,
                    bias=maskT_t[:, b : b + 1],
                    scale=ncinv[:, b : b + 1],
                )

            def stageCS(b):
                s = J * b
                nc.tensor.matmul(
                    out=cs_ps[:J, b : b + 1],
                    lhsT=attnT[:, s : s + J],
                    rhs=ones_t[:M, :],
                    start=True, stop=True,
                )

            def stageB(b, buf):
                s = J * b
                o_ps = ps_o.tile([J, E], f32, tag="o", space="PSUM")
                nc.tensor.matmul(
                    out=o_ps[:, :],
                    lhsT=attnT[:, s : s + J],
                    rhs=bu_all[:, E * b : E * (b + 1)],
                    start=True, stop=True,
                )
                k = b % 8
                if b % 2 == 0:
                    nc.vector.tensor_scalar_mul(
                        buf[:, E * k : E * (k + 1)], o_ps[:, :],
                        rs_all[:J, b : b + 1],
                    )
                else:
                    nc.scalar.activation(
                        out=buf[:, E * k : E * (k + 1)], in_=o_ps[:, :],
                        func=AF.Identity, scale=rs_all[:J, b : b + 1],
                    )

            def flush(b0, n, buf):
                k0 = (b0 % 8) * E
                dst = bass.AP(
                    tensor=out.tensor,
                    offset=b0 * J * E,
                    ap=[[E, J], [J * E, n], [1, E]],
                )
                nc.sync.dma_start(out=dst, in_=buf[:, k0 : k0 + n * E])

            fbase = [0]

            def runB():
                nonlocal obuf
                b = pendB.pop(0)
                if obuf is None:
                    obuf = outp.tile([J, 8 * E], f32, tag="ob")
                    fbase[0] = b
                stageB(b, obuf)
                n = b - fbase[0] + 1
                if n == 8 or b == BLOC - 1 or (
                        b >= BLOC - 8 and (b + 1) % 4 == 0):
                    flush(fbase[0], n, obuf)
                    obuf = None

            def window(lo, hi):
                # ntinv for batches [lo, hi), then their softmax chain
                n = hi - lo
                nc.scalar.activation(
                    out=lnt_t[:J, lo:hi], in_=nt_ps[:J, lo:hi],
                    func=AF.Ln, bias=eps_t[:J],
                )
                nc.scalar.activation(
                    out=ntinv[:J, lo:hi], in_=lnt_t[:J, lo:hi],
                    func=AF.Exp, scale=-0.5,
                )
                for b in range(lo, hi):
                    stageA(b)
                    if pendB:
                        runB()
                for b in range(lo, hi):
                    stageCS(b)
                nc.vector.reciprocal(
                    out=rs_all[:J, lo:hi], in_=cs_ps[:J, lo:hi],
                )
                pendB.extend(range(lo, hi))

            done_b = 0   # batches whose nt-norm matmul has been emitted
            blk0 = 0
            GSIZES = [8, 8, 8, 8, 8, 4, 4, 1, 1, 1]
            assert sum(GSIZES) == NTWB
            for nblk in GSIZES:
                floor_ms = (3000 + 1038 * (NUB + blk0 + nblk)) * 1e-6
                with tc.tile_wait_until(floor_ms):
                    tr = ps_tr.tile([128, 1024], bf16, tag="tr", space="PSUM")
                    for i in range(nblk):
                        nc.tensor.transpose(
                            out=tr[:D, 128 * i : 128 * (i + 1)],
                            in_=twg[:, blk0 + i, :],
                            identity=eye_t[:, :],
                        )
                    s2 = 128 * blk0
                    nc.vector.tensor_copy(
                        out=tpT_all[:, s2 : s2 + 128 * nblk],
                        in_=tr[:D, : 128 * nblk],
                    )
                    nc.vector.tensor_mul(
                        out=tp2_all[:, s2 : s2 + 128 * nblk],
                        in0=tpT_all[:, s2 : s2 + 128 * nblk],
                        in1=tpT_all[:, s2 : s2 + 128 * nblk],
                    )
                    blk0 += nblk
                    nb = min(BLOC, (128 * blk0) // J)
                    for b in range(done_b, nb):
                        nc.tensor.matmul(
                            out=nt_ps[:J, b : b + 1],
                            lhsT=tp2_all[:, J * b : J * (b + 1)],
                            rhs=ones_t[:D, :],
                            start=True, stop=True,
                        )
                    if nb > done_b:
                        window(done_b, nb)
                    done_b = nb
            while pendB:
                runB()

    if NOPATCH:
        nc.compile()
    else:
        orig = bacc_mod.get_activation_tables
        bacc_mod.get_activation_tables = _patched_tables(orig)
        try:
            nc.compile()
        finally:
            bacc_mod.get_activation_tables = orig
    return nc


def _get_program():
    if "nc" not in _CACHE:
        _CACHE["nc"] = _build_program()
    return _CACHE["nc"]


def _prep_inputs(batch_titems, batch_citems, batch_pad_ids, t_emb, c_emb,
                 Ac_w, Ac_b, At_w, At_b, Bc_w, Bc_b, R_w, R_b):
    import ml_dtypes
    bf16 = ml_dtypes.bfloat16

    f = lambda x: np.ascontiguousarray(np.asarray(x, dtype=np.float32))
    t32 = np.asarray(t_emb, np.float32)
    c32 = np.asarray(c_emb, np.float32)
    At = np.asarray(At_w, np.float32)
    Ac = np.asarray(Ac_w, np.float32)
    Bc = np.asarray(Bc_w, np.float32)
    R = np.asarray(R_w, np.float32)

    # tw rows = t_emb @ At^T + At_b (bias folded into the table)
    twe = np.ascontiguousarray(
        (t32 @ At.T + np.asarray(At_b, np.float32)).astype(bf16))
    cemb_b = np.ascontiguousarray(c32.astype(bf16))

    tit = np.asarray(batch_titems).astype(np.int32)
    cit = np.asarray(batch_citems).astype(np.int32)
    pad = np.asarray(batch_pad_ids).astype(np.int64)

    mask = np.zeros((B, M), np.float32)
    mask[pad[0], pad[1]] = NEG

    acw = np.ascontiguousarray(Ac.T.astype(bf16))
    wrb = np.ascontiguousarray((R @ Bc).T.astype(bf16))
    acb = f(np.asarray(Ac_b).reshape(D, 1))
    rbeff = f(
        (np.asarray(R_b, np.float32) + R @ np.asarray(Bc_b, np.float32)
         ).reshape(1, E)
    )
    eye = np.eye(128, dtype=np.float32).astype(bf16)

    in_maps = []
    for c in range(NCORES):
        s = c * BLOC
        # tight tw offsets: item i = (b*J + j) at partition i%128, block i//128
        tflat = tit[s : s + BLOC].reshape(-1)  # [6464]
        tpad = np.zeros(NTWB * 128, np.int32)
        tpad[:NTW] = tflat
        offt = np.ascontiguousarray(tpad.reshape(NTWB, 128).T)

        cflat = cit[s : s + BLOC].reshape(-1)  # [3200]
        offc = np.ascontiguousarray(cflat.reshape(NUB, 128).T)

        maskTc = np.ascontiguousarray(mask[s : s + BLOC].T)  # [50,64]

        in_maps.append(
            {
                "twe": twe,
                "cemb": cemb_b,
                "acw": acw,
                "wrb": wrb,
                "acb": acb,
                "rbeff": rbeff,
                "eye": eye,
                "offt": offt,
                "offc": offc,
                "maskT": maskTc,
            }
        )
    return in_maps


def run_sharded(in_maps, **kwargs):
    from concourse.bass_utils import run_bass_kernel_spmd

    nc = _get_program()
    res = run_bass_kernel_spmd(nc, in_maps, core_ids=list(range(NCORES)), **kwargs)
    outs = [res.results[c]["out"] for c in range(NCORES)]
    full = np.concatenate(outs, axis=0)
    return full, res


def kernel(**inputs):
    in_maps = _prep_inputs(**inputs)
    full, _ = run_sharded(in_maps)
    return full.astype(np.float32)


# revision 49
# speedup vs baseline: 1.0221x; 1.0221x over previous
"""AttentiveItemToVec Trainium2 kernel (batched bf16, host-folded weights).

Full-input contract: kernel(**inputs) takes the unsharded numpy inputs and
returns the full [512, 101, 128] float32 output. Internally shards the batch
across 8 NeuronCores (64 batches each), runs a Bass/Tile kernel per core via
run_bass_kernel_spmd, and concatenates the per-core outputs.

Host prep folds weight-space linear maps (data-independent):
  tw = t_emb @ At_w^T + At_b  [V, 60] -> the gather IS the target projection
  W  = (R_w Bc_w)^T                   -> R folded into Bc (one output matmul)
  rbeff = R_b + R_w @ Bc_b            -> folded into the bu rows on device

Device (per core, 64 batches):
- Gathers use indirect DMA at its max of 128 rows/instruction (this
  runtime generates one descriptor per partition): 51 tight tw blocks +
  25 tight raw-u blocks = 76 instructions saturating the GpSimd queue;
  everything else is sized to hide under it.
- PE transposes flip gathered blocks feature-major (8 blocks per PSUM
  bank, one DVE 2x copy per bank). Context projection cp = uT^T Ac^T runs
  in 512-col chunks, bias folded into the PSUM->SBUF copy. bu = u W^T runs
  per batch (windowed lhsT keeps outputs at partition 0) with rbeff added
  during the grouped PSUM->SBUF copy.
- Norms: per-batch matmul-with-ones into shared PSUM columns, then 4
  batched Ln/Exp activations for 1/|x| = exp(-0.5*ln(x^2+eps)).
- Per-batch softmax chain: dot matmul -> DVE row scale (ntinv) -> PE
  transpose -> Act exp (scale=ncinv, bias=pad mask) -> cs matmul into a
  shared column. Group reciprocal per 16 batches; output stage (o matmul,
  then a pure rsinv scale split across DVE/Act) runs one group behind,
  flushed to HBM in 8-batch DMAs.
"""

import numpy as np
from contextlib import ExitStack

# Problem constants (hardcoded per contract).
V, E, D = 100000, 128, 60
B, J, M, P = 512, 101, 50, 5120
NCORES = 8
BLOC = B // NCORES          # 64 batches per core
NTW = BLOC * J              # 6464 target rows per core
NU = BLOC * M               # 3200 context rows per core
NTWB = (NTW + 127) // 128   # 51 tight tw blocks
NUB = NU // 128             # 25 tight u blocks
NEG = -1.0e30
EPS2 = 1e-12

_CACHE = {}

_ACT_TABLE = "natural_log_exp_and_others"


def _patched_tables(orig_fn):
    def fn(arch):
        tabs = orig_fn(arch)
        return {
            name: (s if name == _ACT_TABLE else type(s)())
            for name, s in tabs.items()
        }
    return fn


def _build_program():
    import os
    NOPATCH = os.environ.get("K_NOPATCH") == "1"
    import concourse.bass as bass
    import concourse.tile as tile
    import concourse.bacc as bacc_mod
    from concourse import bacc, mybir

    f32 = mybir.dt.float32
    bf16 = mybir.dt.bfloat16
    i32 = mybir.dt.int32

    nc = bacc.Bacc(
        "TRN2",
        target_bir_lowering=False,
        debug=False,
        enable_asserts=False,
    )

    twe = nc.dram_tensor("twe", [V, D], bf16, kind="ExternalInput").ap()
    cemb = nc.dram_tensor("cemb", [V, E], bf16, kind="ExternalInput").ap()
    acw = nc.dram_tensor("acw", [E, D], bf16, kind="ExternalInput").ap()
    wrb = nc.dram_tensor("wrb", [E, E], bf16, kind="ExternalInput").ap()
    acb = nc.dram_tensor("acb", [D, 1], f32, kind="ExternalInput").ap()
    rbeff = nc.dram_tensor("rbeff", [1, E], f32, kind="ExternalInput").ap()
    eye = nc.dram_tensor("eye", [128, 128], bf16, kind="ExternalInput").ap()
    offt = nc.dram_tensor("offt", [128, NTWB], i32, kind="ExternalInput").ap()
    offc = nc.dram_tensor("offc", [128, NUB], i32, kind="ExternalInput").ap()
    maskT = nc.dram_tensor("maskT", [M, BLOC], f32, kind="ExternalInput").ap()
    out = nc.dram_tensor("out", [BLOC, J, E], f32, kind="ExternalOutput").ap()

    AF = mybir.ActivationFunctionType
    ALU = mybir.AluOpType

    with tile.TileContext(nc) as tc, ExitStack() as ctx:
        const = ctx.enter_context(tc.tile_pool(name="const", bufs=1))
        big = ctx.enter_context(tc.tile_pool(name="big", bufs=1))
        outp = ctx.enter_context(tc.tile_pool(name="outp", bufs=3))
        dotp = ctx.enter_context(tc.tile_pool(name="dotp", bufs=8))

        # --- constants (gather offsets first so the SWDGE stream starts early)
        offc_t = const.tile([128, NUB], i32)
        nc.sync.dma_start(out=offc_t[:], in_=offc[:, :])
        offt_t = const.tile([128, NTWB], i32)
        nc.sync.dma_start(out=offt_t[:], in_=offt[:, :])
        eye_t = const.tile([128, 128], bf16)
        nc.sync.dma_start(out=eye_t[:], in_=eye[:, :])
        acw_t = const.tile([E, D], bf16)
        nc.sync.dma_start(out=acw_t[:], in_=acw[:, :])
        wrb_t = const.tile([E, E], bf16)
        nc.sync.dma_start(out=wrb_t[:], in_=wrb[:, :])
        acb_t = const.tile([D, 1], f32)
        nc.sync.dma_start(out=acb_t[:], in_=acb[:, :])
        rb4_t = const.tile([M, 4 * E], f32)
        rb4_bcast = bass.AP(tensor=rbeff.tensor, offset=0,
                            ap=[[0, M], [0, 4], [1, E]])
        rb4_dst = rb4_t[:]
        nc.sync.dma_start(
            out=bass.AP(tensor=rb4_dst.tensor, offset=rb4_dst.offset,
                        ap=[rb4_dst.ap[0], [E, 4], [1, E]]),
            in_=rb4_bcast,
        )
        maskT_t = const.tile([M, BLOC], f32)
        nc.sync.dma_start(out=maskT_t[:], in_=maskT[:, :])
        ones_t = const.tile([128, 1], bf16)
        nc.vector.memset(ones_t[:], 1.0)
        eps_t = const.tile([128, 1], f32)
        nc.vector.memset(eps_t[:], EPS2)

        # --- big SBUF arrays ---
        twg = big.tile([128, NTWB, D], bf16)     # gathered tw rows, tight
        ug = big.tile([128, NUB, E], bf16)       # gathered c_emb rows, tight
        tpT_all = big.tile([D, NTWB * 128], bf16)  # feature-major tp
        uT_all = big.tile([E, NU], bf16)         # feature-major u
        cpT_all = big.tile([D, NU], bf16)        # context projection (+Ac_b)
        tp2_all = big.tile([D, NTWB * 128], bf16)
        cp2_all = big.tile([D, NU], bf16)
        bu_all = big.tile([M, BLOC * E], bf16)   # u @ (R Bc)^T + rbeff
        attnT = big.tile([M, BLOC * J], bf16)
        ntinv = big.tile([128, BLOC], f32)
        ncinv = big.tile([M, BLOC], f32)
        lnt_t = big.tile([128, BLOC], f32)
        lnc_t = big.tile([M, BLOC], f32)
        rs_all = big.tile([128, BLOC], f32)

        # --- gathers: 128 rows per indirect DMA (one desc per partition).
        # u blocks first (they feed the early u-side pipeline), then tw.
        for c in range(NUB):
            nc.gpsimd.indirect_dma_start(
                out=ug[:, c, :], out_offset=None, in_=cemb[:, :],
                in_offset=bass.IndirectOffsetOnAxis(
                    ap=offc_t[:, c : c + 1], axis=0
                ),
            )
        for c in range(NTWB):
            nc.gpsimd.indirect_dma_start(
                out=twg[:, c, :], out_offset=None, in_=twe[:, :],
                in_offset=bass.IndirectOffsetOnAxis(
                    ap=offt_t[:, c : c + 1], axis=0
                ),
            )

        if True:
            ps_tr = ctx.enter_context(
                tc.tile_pool(name="ps_tr", bufs=1, space="PSUM"))
            ps_sm = ctx.enter_context(
                tc.tile_pool(name="ps_sm", bufs=1, space="PSUM"))

            # nt columns and cs columns share one PSUM bank
            sm_ps = ps_sm.tile([128, 128], f32, tag="sm", space="PSUM")
            nt_ps = sm_ps[:, :BLOC]
            cs_ps = sm_ps[:, BLOC : 2 * BLOC]

            # ---- u side: transposes, projection, bu, context norms ----
            with tc.tile_pool(name="ps_pj", bufs=2, space="PSUM") as ps_pj, \
                 tc.tile_pool(name="ps_nc", bufs=1, space="PSUM") as ps_nc:
                for g in range((NUB + 7) // 8):
                    nblk = min(8, NUB - 8 * g)
                    floor_ms = (3000 + 1038 * (8 * g + nblk)) * 1e-6
                    with tc.tile_wait_until(floor_ms):
                        tr = ps_tr.tile([128, 1024], bf16, tag="tr",
                                        space="PSUM")
                        for i in range(nblk):
                            nc.tensor.transpose(
                                out=tr[:, 128 * i : 128 * (i + 1)],
                                in_=ug[:, 8 * g + i, :],
                                identity=eye_t[:, :],
                            )
                        nc.vector.tensor_copy(
                            out=uT_all[:, 1024 * g : 1024 * g + 128 * nblk],
                            in_=tr[:, : 128 * nblk],
                        )
                for c in range((NU + 511) // 512):
                    s = 512 * c
                    w = min(512, NU - s)
                    pj = ps_pj.tile([128, 512], f32, tag="pj", space="PSUM")
                    nc.tensor.matmul(
                        out=pj[:D, :w], lhsT=acw_t[:], rhs=uT_all[:, s : s + w],
                        start=True, stop=True,
                    )
                    if c % 2 == 0:
                        nc.vector.tensor_scalar_add(
                            out=cpT_all[:, s : s + w], in0=pj[:D, :w],
                            scalar1=acb_t[:],
                        )
                    else:
                        nc.scalar.activation(
                            out=cpT_all[:, s : s + w], in_=pj[:D, :w],
                            func=AF.Identity, bias=acb_t[:], scale=1.0,
                        )
                    nc.vector.tensor_mul(
                        out=cp2_all[:, s : s + w],
                        in0=cpT_all[:, s : s + w],
                        in1=cpT_all[:, s : s + w],
                    )
                nc_ps = ps_nc.tile([M, BLOC], f32, tag="ncn", space="PSUM")
                for b in range(BLOC):
                    nc.tensor.matmul(
                        out=nc_ps[:, b : b + 1],
                        lhsT=cp2_all[:, M * b : M * (b + 1)],
                        rhs=ones_t[:D, :],
                        start=True, stop=True,
                    )
                nc.scalar.activation(
                    out=lnc_t[:], in_=nc_ps[:, :], func=AF.Ln, bias=eps_t[:M]
                )
                nc.scalar.activation(
                    out=ncinv[:], in_=lnc_t[:], func=AF.Exp, scale=-0.5
                )
                for c in range(BLOC // 4):
                    bu_ps = ps_pj.tile([128, 512], f32, tag="pj", space="PSUM")
                    for i in range(4):
                        b = 4 * c + i
                        nc.tensor.matmul(
                            out=bu_ps[:M, 128 * i : 128 * (i + 1)],
                            lhsT=uT_all[:, M * b : M * b + M],
                            rhs=wrb_t[:],
                            start=True, stop=True,
                        )
                    nc.vector.scalar_tensor_tensor(
                        out=bu_all[:, 512 * c : 512 * (c + 1)],
                        in0=bu_ps[:M, :], scalar=1.0,
                        in1=rb4_t[:],
                        op0=ALU.mult, op1=ALU.add,
                    )

            # ---- tw side + attention, interleaved with the gather stream ----
            # One PSUM bank per in-flight batch: dot at byte 0 (f32 [101,50]),
            # dotT at byte 1024 (bf16 [50,128] via bitcast). 4 banks = depth 4.
            ps_ch = ctx.enter_context(
                tc.tile_pool(name="ps_ch", bufs=4, space="PSUM"))
            ps_o = ctx.enter_context(
                tc.tile_pool(name="ps_o", bufs=2, space="PSUM"))
            GRP = 16
            pendB = []
            obuf = None

            def stageA(b):
                s = J * b
                ch = ps_ch.tile([128, 512], f32, tag="ch", space="PSUM")
                dot_ps = ch[:J, :M]
                nc.tensor.matmul(
                    out=dot_ps,
                    lhsT=tpT_all[:, s : s + J],
                    rhs=cpT_all[:, M * b : M * b + M],
                    start=True, stop=True,
                )
                dotn = dotp.tile([J, M], bf16, tag="dotn")
                nc.vector.tensor_scalar_mul(
                    dotn[:], dot_ps, ntinv[:J, b : b + 1]
                )
                dT_ps = ch[:M, 256:320].bitcast(bf16)
                nc.tensor.transpose(
                    out=dT_ps[:, :J], in_=dotn[:], identity=eye_t[:J, :J]
                )
                nc.scalar.activation(
                    out=attnT[:, s : s + J],
                    in_=dT_ps[:, :J],
                    func=AF.Exp,
                    bias=maskT_t[:, b : b + 1],
                    scale=ncinv[:, b : b + 1],
                )

            def stageCS(b):
                s = J * b
                nc.tensor.matmul(
                    out=cs_ps[:J, b : b + 1],
                    lhsT=attnT[:, s : s + J],
                    rhs=ones_t[:M, :],
                    start=True, stop=True,
                )

            def stageB(b, buf):
                s = J * b
                o_ps = ps_o.tile([J, E], f32, tag="o", space="PSUM")
                nc.tensor.matmul(
                    out=o_ps[:, :],
                    lhsT=attnT[:, s : s + J],
                    rhs=bu_all[:, E * b : E * (b + 1)],
                    start=True, stop=True,
                )
                k = b % 8
                if True:
                    nc.vector.tensor_scalar_mul(
                        buf[:, E * k : E * (k + 1)], o_ps[:, :],
                        rs_all[:J, b : b + 1],
                    )
                else:
                    nc.scalar.activation(
                        out=buf[:, E * k : E * (k + 1)], in_=o_ps[:, :],
                        func=AF.Identity, scale=rs_all[:J, b : b + 1],
                    )

            def flush(b0, n, buf):
                k0 = (b0 % 8) * E
                dst = bass.AP(
                    tensor=out.tensor,
                    offset=b0 * J * E,
                    ap=[[E, J], [J * E, n], [1, E]],
                )
                nc.sync.dma_start(out=dst, in_=buf[:, k0 : k0 + n * E])

            fbase = [0]

            def runB():
                nonlocal obuf
                b = pendB.pop(0)
                if obuf is None:
                    obuf = outp.tile([J, 8 * E], f32, tag="ob")
                    fbase[0] = b
                stageB(b, obuf)
                n = b - fbase[0] + 1
                if n == 8 or b == BLOC - 1 or (
                        b >= BLOC - 8 and (b + 1) % 4 == 0):
                    flush(fbase[0], n, obuf)
                    obuf = None

            def window(lo, hi):
                # ntinv for batches [lo, hi), then their softmax chain
                n = hi - lo
                nc.scalar.activation(
                    out=lnt_t[:J, lo:hi], in_=nt_ps[:J, lo:hi],
                    func=AF.Ln, bias=eps_t[:J],
                )
                nc.scalar.activation(
                    out=ntinv[:J, lo:hi], in_=lnt_t[:J, lo:hi],
                    func=AF.Exp, scale=-0.5,
                )
                for b in range(lo, hi):
                    stageA(b)
                    if pendB:
                        runB()
                for b in range(lo, hi):
                    stageCS(b)
                nc.vector.reciprocal(
                    out=rs_all[:J, lo:hi], in_=cs_ps[:J, lo:hi],
                )
                pendB.extend(range(lo, hi))

            done_b = 0   # batches whose nt-norm matmul has been emitted
            blk0 = 0
            GSIZES = [8, 8, 8, 8, 8, 4, 4, 1, 1, 1]
            assert sum(GSIZES) == NTWB
            for nblk in GSIZES:
                floor_ms = (3000 + 1038 * (NUB + blk0 + nblk)) * 1e-6
                with tc.tile_wait_until(floor_ms):
                    tr = ps_tr.tile([128, 1024], bf16, tag="tr", space="PSUM")
                    for i in range(nblk):
                        nc.tensor.transpose(
                            out=tr[:D, 128 * i : 128 * (i + 1)],
                            in_=twg[:, blk0 + i, :],
                            identity=eye_t[:, :],
                        )
                    s2 = 128 * blk0
                    nc.vector.tensor_copy(
                        out=tpT_all[:, s2 : s2 + 128 * nblk],
                        in_=tr[:D, : 128 * nblk],
                    )
                    nc.vector.tensor_mul(
                        out=tp2_all[:, s2 : s2 + 128 * nblk],
                        in0=tpT_all[:, s2 : s2 + 128 * nblk],
                        in1=tpT_all[:, s2 : s2 + 128 * nblk],
                    )
                    blk0 += nblk
                    nb = min(BLOC, (128 * blk0) // J)
                    for b in range(done_b, nb):
                        nc.tensor.matmul(
                            out=nt_ps[:J, b : b + 1],
                            lhsT=tp2_all[:, J * b : J * (b + 1)],
                            rhs=ones_t[:D, :],
                            start=True, stop=True,
                        )
                    if nb > done_b:
                        window(done_b, nb)
                    done_b = nb
            while pendB:
                runB()

    if NOPATCH:
        nc.compile()
    else:
        orig = bacc_mod.get_activation_tables
        bacc_mod.get_activation_tables = _patched_tables(orig)
        try:
            nc.compile()
        finally:
            bacc_mod.get_activation_tables = orig
    return nc


def _get_program():
    if "nc" not in _CACHE:
        _CACHE["nc"] = _build_program()
    return _CACHE["nc"]


def _prep_inputs(batch_titems, batch_citems, batch_pad_ids, t_emb, c_emb,
                 Ac_w, Ac_b, At_w, At_b, Bc_w, Bc_b, R_w, R_b):
    import ml_dtypes
    bf16 = ml_dtypes.bfloat16

    f = lambda x: np.ascontiguousarray(np.asarray(x, dtype=np.float32))
    t32 = np.asarray(t_emb, np.float32)
    c32 = np.asarray(c_emb, np.float32)
    At = np.asarray(At_w, np.float32)
    Ac = np.asarray(Ac_w, np.float32)
    Bc = np.asarray(Bc_w, np.float32)
    R = np.asarray(R_w, np.float32)

    # tw rows = t_emb @ At^T + At_b (bias folded into the table)
    twe = np.ascontiguousarray(
        (t32 @ At.T + np.asarray(At_b, np.float32)).astype(bf16))
    cemb_b = np.ascontiguousarray(c32.astype(bf16))

    tit = np.asarray(batch_titems).astype(np.int32)
    cit = np.asarray(batch_citems).astype(np.int32)
    pad = np.asarray(batch_pad_ids).astype(np.int64)

    mask = np.zeros((B, M), np.float32)
    mask[pad[0], pad[1]] = NEG

    acw = np.ascontiguousarray(Ac.T.astype(bf16))
    wrb = np.ascontiguousarray((R @ Bc).T.astype(bf16))
    acb = f(np.asarray(Ac_b).reshape(D, 1))
    rbeff = f(
        (np.asarray(R_b, np.float32) + R @ np.asarray(Bc_b, np.float32)
         ).reshape(1, E)
    )
    eye = np.eye(128, dtype=np.float32).astype(bf16)

    in_maps = []
    for c in range(NCORES):
        s = c * BLOC
        # tight tw offsets: item i = (b*J + j) at partition i%128, block i//128
        tflat = tit[s : s + BLOC].reshape(-1)  # [6464]
        tpad = np.zeros(NTWB * 128, np.int32)
        tpad[:NTW] = tflat
        offt = np.ascontiguousarray(tpad.reshape(NTWB, 128).T)

        cflat = cit[s : s + BLOC].reshape(-1)  # [3200]
        offc = np.ascontiguousarray(cflat.reshape(NUB, 128).T)

        maskTc = np.ascontiguousarray(mask[s : s + BLOC].T)  # [50,64]

        in_maps.append(
            {
                "twe": twe,
                "cemb": cemb_b,
                "acw": acw,
                "wrb": wrb,
                "acb": acb,
                "rbeff": rbeff,
                "eye": eye,
                "offt": offt,
                "offc": offc,
                "maskT": maskTc,
            }
        )
    return in_maps


def run_sharded(in_maps, **kwargs):
    from concourse.bass_utils import run_bass_kernel_spmd

    nc = _get_program()
    res = run_bass_kernel_spmd(nc, in_maps, core_ids=list(range(NCORES)), **kwargs)
    outs = [res.results[c]["out"] for c in range(NCORES)]
    full = np.concatenate(outs, axis=0)
    return full, res


def kernel(**inputs):
    in_maps = _prep_inputs(**inputs)
    full, _ = run_sharded(in_maps)
    return full.astype(np.float32)


# revision 53
# speedup vs baseline: 1.0257x; 1.0035x over previous
"""AttentiveItemToVec Trainium2 kernel (batched bf16, host-folded weights).

Full-input contract: kernel(**inputs) takes the unsharded numpy inputs and
returns the full [512, 101, 128] float32 output. Internally shards the batch
across 8 NeuronCores (64 batches each), runs a Bass/Tile kernel per core via
run_bass_kernel_spmd, and concatenates the per-core outputs.

Host prep folds weight-space linear maps (data-independent):
  tw = t_emb @ At_w^T + At_b  [V, 60] -> the gather IS the target projection
  W  = (R_w Bc_w)^T                   -> R folded into Bc (one output matmul)
  rbeff = R_b + R_w @ Bc_b            -> folded into the bu rows on device

Device (per core, 64 batches):
- Gathers use indirect DMA at its max of 128 rows/instruction (this
  runtime generates one descriptor per partition): 51 tight tw blocks +
  25 tight raw-u blocks = 76 instructions saturating the GpSimd queue;
  everything else is sized to hide under it.
- PE transposes flip gathered blocks feature-major (8 blocks per PSUM
  bank, one DVE 2x copy per bank). Context projection cp = uT^T Ac^T runs
  in 512-col chunks, bias folded into the PSUM->SBUF copy. bu = u W^T runs
  per batch (windowed lhsT keeps outputs at partition 0) with rbeff added
  during the grouped PSUM->SBUF copy.
- Norms: per-batch matmul-with-ones into shared PSUM columns, then 4
  batched Ln/Exp activations for 1/|x| = exp(-0.5*ln(x^2+eps)).
- Per-batch softmax chain: dot matmul -> DVE row scale (ntinv) -> PE
  transpose -> Act exp (scale=ncinv, bias=pad mask) -> cs matmul into a
  shared column. Group reciprocal per 16 batches; output stage (o matmul,
  then a pure rsinv scale split across DVE/Act) runs one group behind,
  flushed to HBM in 8-batch DMAs.
"""

import numpy as np
from contextlib import ExitStack

# Problem constants (hardcoded per contract).
V, E, D = 100000, 128, 60
B, J, M, P = 512, 101, 50, 5120
NCORES = 8
BLOC = B // NCORES          # 64 batches per core
NTW = BLOC * J              # 6464 target rows per core
NU = BLOC * M               # 3200 context rows per core
NTWB = (NTW + 127) // 128   # 51 tight tw blocks
NUB = NU // 128             # 25 tight u blocks
NEG = -1.0e30
EPS2 = 1e-12

_CACHE = {}

_ACT_TABLE = "natural_log_exp_and_others"


def _patched_tables(orig_fn):
    def fn(arch):
        tabs = orig_fn(arch)
        return {
            name: (s if name == _ACT_TABLE else type(s)())
            for name, s in tabs.items()
        }
    return fn


def _build_program():
    import os
    NOPATCH = os.environ.get("K_NOPATCH") == "1"
    import concourse.bass as bass
    import concourse.tile as tile
    import concourse.bacc as bacc_mod
    from concourse import bacc, mybir

    f32 = mybir.dt.float32
    bf16 = mybir.dt.bfloat16
    i32 = mybir.dt.int32

    nc = bacc.Bacc(
        "TRN2",
        target_bir_lowering=False,
        debug=False,
        enable_asserts=False,
    )

    twe = nc.dram_tensor("twe", [V, D], bf16, kind="ExternalInput").ap()
    cemb = nc.dram_tensor("cemb", [V, E], bf16, kind="ExternalInput").ap()
    acw = nc.dram_tensor("acw", [E, D], bf16, kind="ExternalInput").ap()
    wrb = nc.dram_tensor("wrb", [E, E], bf16, kind="ExternalInput").ap()
    acb = nc.dram_tensor("acb", [D, 1], f32, kind="ExternalInput").ap()
    rbeff = nc.dram_tensor("rbeff", [1, E], f32, kind="ExternalInput").ap()
    eye = nc.dram_tensor("eye", [128, 128], bf16, kind="ExternalInput").ap()
    offt = nc.dram_tensor("offt", [128, NTWB], i32, kind="ExternalInput").ap()
    offc = nc.dram_tensor("offc", [128, NUB], i32, kind="ExternalInput").ap()
    maskT = nc.dram_tensor("maskT", [M, BLOC], f32, kind="ExternalInput").ap()
    out = nc.dram_tensor("out", [BLOC, J, E], f32, kind="ExternalOutput").ap()

    AF = mybir.ActivationFunctionType
    ALU = mybir.AluOpType

    with tile.TileContext(nc) as tc, ExitStack() as ctx:
        const = ctx.enter_context(tc.tile_pool(name="const", bufs=1))
        big = ctx.enter_context(tc.tile_pool(name="big", bufs=1))
        outp = ctx.enter_context(tc.tile_pool(name="outp", bufs=4))
        dotp = ctx.enter_context(tc.tile_pool(name="dotp", bufs=12))

        # --- constants (gather offsets first so the SWDGE stream starts early)
        offc_t = const.tile([128, NUB], i32)
        nc.sync.dma_start(out=offc_t[:], in_=offc[:, :])
        offt_t = const.tile([128, NTWB], i32)
        nc.sync.dma_start(out=offt_t[:], in_=offt[:, :])
        eye_t = const.tile([128, 128], bf16)
        nc.sync.dma_start(out=eye_t[:], in_=eye[:, :])
        acw_t = const.tile([E, D], bf16)
        nc.sync.dma_start(out=acw_t[:], in_=acw[:, :])
        wrb_t = const.tile([E, E], bf16)
        nc.sync.dma_start(out=wrb_t[:], in_=wrb[:, :])
        acb_t = const.tile([D, 1], f32)
        nc.sync.dma_start(out=acb_t[:], in_=acb[:, :])
        rb4_t = const.tile([M, 4 * E], f32)
        rb4_bcast = bass.AP(tensor=rbeff.tensor, offset=0,
                            ap=[[0, M], [0, 4], [1, E]])
        rb4_dst = rb4_t[:]
        nc.sync.dma_start(
            out=bass.AP(tensor=rb4_dst.tensor, offset=rb4_dst.offset,
                        ap=[rb4_dst.ap[0], [E, 4], [1, E]]),
            in_=rb4_bcast,
        )
        maskT_t = const.tile([M, BLOC], f32)
        nc.sync.dma_start(out=maskT_t[:], in_=maskT[:, :])
        ones_t = const.tile([128, 1], bf16)
        nc.vector.memset(ones_t[:], 1.0)
        eps_t = const.tile([128, 1], f32)
        nc.vector.memset(eps_t[:], EPS2)

        # --- big SBUF arrays ---
        twg = big.tile([128, NTWB, D], bf16)     # gathered tw rows, tight
        ug = big.tile([128, NUB, E], bf16)       # gathered c_emb rows, tight
        tpT_all = big.tile([D, NTWB * 128], bf16)  # feature-major tp
        uT_all = big.tile([E, NU], bf16)         # feature-major u
        cpT_all = big.tile([D, NU], bf16)        # context projection (+Ac_b)
        tp2_all = big.tile([D, NTWB * 128], bf16)
        cp2_all = big.tile([D, NU], bf16)
        bu_all = big.tile([M, BLOC * E], bf16)   # u @ (R Bc)^T + rbeff
        attnT = big.tile([M, BLOC * J], bf16)
        ntinv = big.tile([128, BLOC], f32)
        ncinv = big.tile([M, BLOC], f32)
        lnt_t = big.tile([128, BLOC], f32)
        lnc_t = big.tile([M, BLOC], f32)
        rs_all = big.tile([128, BLOC], f32)

        # --- gathers: 128 rows per indirect DMA (one desc per partition).
        # u blocks first (they feed the early u-side pipeline), then tw.
        for c in range(NUB):
            nc.gpsimd.indirect_dma_start(
                out=ug[:, c, :], out_offset=None, in_=cemb[:, :],
                in_offset=bass.IndirectOffsetOnAxis(
                    ap=offc_t[:, c : c + 1], axis=0
                ),
            )
        for c in range(NTWB):
            nc.gpsimd.indirect_dma_start(
                out=twg[:, c, :], out_offset=None, in_=twe[:, :],
                in_offset=bass.IndirectOffsetOnAxis(
                    ap=offt_t[:, c : c + 1], axis=0
                ),
            )

        if True:
            ps_tr = ctx.enter_context(
                tc.tile_pool(name="ps_tr", bufs=1, space="PSUM"))
            ps_sm = ctx.enter_context(
                tc.tile_pool(name="ps_sm", bufs=1, space="PSUM"))

            # nt columns and cs columns share one PSUM bank
            sm_ps = ps_sm.tile([128, 128], f32, tag="sm", space="PSUM")
            nt_ps = sm_ps[:, :BLOC]
            cs_ps = sm_ps[:, BLOC : 2 * BLOC]

            # ---- u side: transposes, projection, bu, context norms ----
            with tc.tile_pool(name="ps_pj", bufs=2, space="PSUM") as ps_pj, \
                 tc.tile_pool(name="ps_nc", bufs=1, space="PSUM") as ps_nc:
                for g in range((NUB + 7) // 8):
                    nblk = min(8, NUB - 8 * g)
                    floor_ms = (3000 + 1038 * (8 * g + nblk)) * 1e-6
                    with tc.tile_wait_until(floor_ms):
                        tr = ps_tr.tile([128, 1024], bf16, tag="tr",
                                        space="PSUM")
                        for i in range(nblk):
                            nc.tensor.transpose(
                                out=tr[:, 128 * i : 128 * (i + 1)],
                                in_=ug[:, 8 * g + i, :],
                                identity=eye_t[:, :],
                            )
                        nc.vector.tensor_copy(
                            out=uT_all[:, 1024 * g : 1024 * g + 128 * nblk],
                            in_=tr[:, : 128 * nblk],
                        )
                for c in range((NU + 511) // 512):
                    s = 512 * c
                    w = min(512, NU - s)
                    pj = ps_pj.tile([128, 512], f32, tag="pj", space="PSUM")
                    nc.tensor.matmul(
                        out=pj[:D, :w], lhsT=acw_t[:], rhs=uT_all[:, s : s + w],
                        start=True, stop=True,
                    )
                    if c % 2 == 0:
                        nc.vector.tensor_scalar_add(
                            out=cpT_all[:, s : s + w], in0=pj[:D, :w],
                            scalar1=acb_t[:],
                        )
                    else:
                        nc.scalar.activation(
                            out=cpT_all[:, s : s + w], in_=pj[:D, :w],
                            func=AF.Identity, bias=acb_t[:], scale=1.0,
                        )
                    nc.vector.tensor_mul(
                        out=cp2_all[:, s : s + w],
                        in0=cpT_all[:, s : s + w],
                        in1=cpT_all[:, s : s + w],
                    )
                nc_ps = ps_nc.tile([M, BLOC], f32, tag="ncn", space="PSUM")
                for b in range(BLOC):
                    nc.tensor.matmul(
                        out=nc_ps[:, b : b + 1],
                        lhsT=cp2_all[:, M * b : M * (b + 1)],
                        rhs=ones_t[:D, :],
                        start=True, stop=True,
                    )
                nc.scalar.activation(
                    out=lnc_t[:], in_=nc_ps[:, :], func=AF.Ln, bias=eps_t[:M]
                )
                nc.scalar.activation(
                    out=ncinv[:], in_=lnc_t[:], func=AF.Exp, scale=-0.5
                )
                for c in range(BLOC // 4):
                    bu_ps = ps_pj.tile([128, 512], f32, tag="pj", space="PSUM")
                    for i in range(4):
                        b = 4 * c + i
                        nc.tensor.matmul(
                            out=bu_ps[:M, 128 * i : 128 * (i + 1)],
                            lhsT=uT_all[:, M * b : M * b + M],
                            rhs=wrb_t[:],
                            start=True, stop=True,
                        )
                    nc.vector.scalar_tensor_tensor(
                        out=bu_all[:, 512 * c : 512 * (c + 1)],
                        in0=bu_ps[:M, :], scalar=1.0,
                        in1=rb4_t[:],
                        op0=ALU.mult, op1=ALU.add,
                    )

            # ---- tw side + attention, interleaved with the gather stream ----
            # One PSUM bank per in-flight batch: dot at byte 0 (f32 [101,50]),
            # dotT at byte 1024 (bf16 [50,128] via bitcast). 4 banks = depth 4.
            ps_ch = ctx.enter_context(
                tc.tile_pool(name="ps_ch", bufs=4, space="PSUM"))
            ps_o = ctx.enter_context(
                tc.tile_pool(name="ps_o", bufs=2, space="PSUM"))
            GRP = 16
            pendB = []
            obuf = None

            def stageA(b):
                s = J * b
                ch = ps_ch.tile([128, 512], f32, tag="ch", space="PSUM")
                dot_ps = ch[:J, :M]
                nc.tensor.matmul(
                    out=dot_ps,
                    lhsT=tpT_all[:, s : s + J],
                    rhs=cpT_all[:, M * b : M * b + M],
                    start=True, stop=True,
                )
                dotn = dotp.tile([J, M], bf16, tag="dotn")
                nc.vector.tensor_scalar_mul(
                    dotn[:], dot_ps, ntinv[:J, b : b + 1]
                )
                dT_ps = ch[:M, 256:320].bitcast(bf16)
                nc.tensor.transpose(
                    out=dT_ps[:, :J], in_=dotn[:], identity=eye_t[:J, :J]
                )
                nc.scalar.activation(
                    out=attnT[:, s : s + J],
                    in_=dT_ps[:, :J],
                    func=AF.Exp,
                    bias=maskT_t[:, b : b + 1],
                    scale=ncinv[:, b : b + 1],
                )

            def stageCS(b):
                s = J * b
                nc.tensor.matmul(
                    out=cs_ps[:J, b : b + 1],
                    lhsT=attnT[:, s : s + J],
                    rhs=ones_t[:M, :],
                    start=True, stop=True,
                )

            def stageB(b, buf):
                s = J * b
                o_ps = ps_o.tile([J, E], f32, tag="o", space="PSUM")
                nc.tensor.matmul(
                    out=o_ps[:, :],
                    lhsT=attnT[:, s : s + J],
                    rhs=bu_all[:, E * b : E * (b + 1)],
                    start=True, stop=True,
                )
                k = b % 8
                if True:
                    nc.vector.tensor_scalar_mul(
                        buf[:, E * k : E * (k + 1)], o_ps[:, :],
                        rs_all[:J, b : b + 1],
                    )
                else:
                    nc.scalar.activation(
                        out=buf[:, E * k : E * (k + 1)], in_=o_ps[:, :],
                        func=AF.Identity, scale=rs_all[:J, b : b + 1],
                    )

            def flush(b0, n, buf):
                k0 = (b0 % 8) * E
                dst = bass.AP(
                    tensor=out.tensor,
                    offset=b0 * J * E,
                    ap=[[E, J], [J * E, n], [1, E]],
                )
                nc.sync.dma_start(out=dst, in_=buf[:, k0 : k0 + n * E])

            fbase = [0]

            def runB():
                nonlocal obuf
                b = pendB.pop(0)
                if obuf is None:
                    obuf = outp.tile([J, 8 * E], f32, tag="ob")
                    fbase[0] = b
                stageB(b, obuf)
                n = b - fbase[0] + 1
                if n == 8 or b == BLOC - 1 or (
                        b >= BLOC - 8 and (b + 1) % 4 == 0) or (
                        b >= BLOC - 4 and (b + 1) % 2 == 0):
                    flush(fbase[0], n, obuf)
                    obuf = None

            def window(lo, hi):
                # ntinv for batches [lo, hi), then their softmax chain
                n = hi - lo
                nc.scalar.activation(
                    out=lnt_t[:J, lo:hi], in_=nt_ps[:J, lo:hi],
                    func=AF.Ln, bias=eps_t[:J],
                )
                nc.scalar.activation(
                    out=ntinv[:J, lo:hi], in_=lnt_t[:J, lo:hi],
                    func=AF.Exp, scale=-0.5,
                )
                for b in range(lo, hi):
                    stageA(b)
                    if pendB:
                        runB()
                for b in range(lo, hi):
                    stageCS(b)
                nc.vector.reciprocal(
                    out=rs_all[:J, lo:hi], in_=cs_ps[:J, lo:hi],
                )
                pendB.extend(range(lo, hi))

            done_b = 0   # batches whose nt-norm matmul has been emitted
            blk0 = 0
            GSIZES = [8, 8, 8, 8, 8, 4, 4, 1, 1, 1]
            assert sum(GSIZES) == NTWB
            for nblk in GSIZES:
                floor_ms = (3000 + 1038 * (NUB + blk0 + nblk)) * 1e-6
                with tc.tile_wait_until(floor_ms):
                    tr = ps_tr.tile([128, 1024], bf16, tag="tr", space="PSUM")
                    for i in range(nblk):
                        nc.tensor.transpose(
                            out=tr[:D, 128 * i : 128 * (i + 1)],
                            in_=twg[:, blk0 + i, :],
                            identity=eye_t[:, :],
                        )
                    s2 = 128 * blk0
                    nc.vector.tensor_copy(
                        out=tpT_all[:, s2 : s2 + 128 * nblk],
                        in_=tr[:D, : 128 * nblk],
                    )
                    nc.vector.tensor_mul(
                        out=tp2_all[:, s2 : s2 + 128 * nblk],
                        in0=tpT_all[:, s2 : s2 + 128 * nblk],
                        in1=tpT_all[:, s2 : s2 + 128 * nblk],
                    )
                    blk0 += nblk
                    nb = min(BLOC, (128 * blk0) // J)
                    for b in range(done_b, nb):
                        nc.tensor.matmul(
                            out=nt_ps[:J, b : b + 1],
                            lhsT=tp2_all[:, J * b : J * (b + 1)],
                            rhs=ones_t[:D, :],
                            start=True, stop=True,
                        )
                    if nb > done_b:
                        window(done_b, nb)
                    done_b = nb
            while pendB:
                runB()

    if NOPATCH:
        nc.compile()
    else:
        orig = bacc_mod.get_activation_tables
        bacc_mod.get_activation_tables = _patched_tables(orig)
        try:
            nc.compile()
        finally:
            bacc_mod.get_activation_tables = orig
    return nc


def _get_program():
    if "nc" not in _CACHE:
        _CACHE["nc"] = _build_program()
    return _CACHE["nc"]


def _prep_inputs(batch_titems, batch_citems, batch_pad_ids, t_emb, c_emb,
                 Ac_w, Ac_b, At_w, At_b, Bc_w, Bc_b, R_w, R_b):
    import ml_dtypes
    bf16 = ml_dtypes.bfloat16

    f = lambda x: np.ascontiguousarray(np.asarray(x, dtype=np.float32))
    t32 = np.asarray(t_emb, np.float32)
    c32 = np.asarray(c_emb, np.float32)
    At = np.asarray(At_w, np.float32)
    Ac = np.asarray(Ac_w, np.float32)
    Bc = np.asarray(Bc_w, np.float32)
    R = np.asarray(R_w, np.float32)

    # tw rows = t_emb @ At^T + At_b (bias folded into the table)
    twe = np.ascontiguousarray(
        (t32 @ At.T + np.asarray(At_b, np.float32)).astype(bf16))
    cemb_b = np.ascontiguousarray(c32.astype(bf16))

    tit = np.asarray(batch_titems).astype(np.int32)
    cit = np.asarray(batch_citems).astype(np.int32)
    pad = np.asarray(batch_pad_ids).astype(np.int64)

    mask = np.zeros((B, M), np.float32)
    mask[pad[0], pad[1]] = NEG

    acw = np.ascontiguousarray(Ac.T.astype(bf16))
    wrb = np.ascontiguousarray((R @ Bc).T.astype(bf16))
    acb = f(np.asarray(Ac_b).reshape(D, 1))
    rbeff = f(
        (np.asarray(R_b, np.float32) + R @ np.asarray(Bc_b, np.float32)
         ).reshape(1, E)
    )
    eye = np.eye(128, dtype=np.float32).astype(bf16)

    in_maps = []
    for c in range(NCORES):
        s = c * BLOC
        # tight tw offsets: item i = (b*J + j) at partition i%128, block i//128
        tflat = tit[s : s + BLOC].reshape(-1)  # [6464]
        tpad = np.zeros(NTWB * 128, np.int32)
        tpad[:NTW] = tflat
        offt = np.ascontiguousarray(tpad.reshape(NTWB, 128).T)

        cflat = cit[s : s + BLOC].reshape(-1)  # [3200]
        offc = np.ascontiguousarray(cflat.reshape(NUB, 128).T)

        maskTc = np.ascontiguousarray(mask[s : s + BLOC].T)  # [50,64]

        in_maps.append(
            {
                "twe": twe,
                "cemb": cemb_b,
                "acw": acw,
                "wrb": wrb,
                "acb": acb,
                "rbeff": rbeff,
                "eye": eye,
                "offt": offt,
                "offc": offc,
                "maskT": maskTc,
            }
        )
    return in_maps


def run_sharded(in_maps, **kwargs):
    from concourse.bass_utils import run_bass_kernel_spmd

    nc = _get_program()
    res = run_bass_kernel_spmd(nc, in_maps, core_ids=list(range(NCORES)), **kwargs)
    outs = [res.results[c]["out"] for c in range(NCORES)]
    full = np.concatenate(outs, axis=0)
    return full, res


def kernel(**inputs):
    in_maps = _prep_inputs(**inputs)
    full, _ = run_sharded(in_maps)
    return full.astype(np.float32)


# revision 54
# speedup vs baseline: 1.0268x; 1.0011x over previous
"""AttentiveItemToVec Trainium2 kernel (batched bf16, host-folded weights).

Full-input contract: kernel(**inputs) takes the unsharded numpy inputs and
returns the full [512, 101, 128] float32 output. Internally shards the batch
across 8 NeuronCores (64 batches each), runs a Bass/Tile kernel per core via
run_bass_kernel_spmd, and concatenates the per-core outputs.

Host prep folds weight-space linear maps (data-independent):
  tw = t_emb @ At_w^T + At_b  [V, 60] -> the gather IS the target projection
  W  = (R_w Bc_w)^T                   -> R folded into Bc (one output matmul)
  rbeff = R_b + R_w @ Bc_b            -> folded into the bu rows on device

Device (per core, 64 batches):
- Gathers use indirect DMA at its max of 128 rows/instruction (this
  runtime generates one descriptor per partition): 51 tight tw blocks +
  25 tight raw-u blocks = 76 instructions saturating the GpSimd queue;
  everything else is sized to hide under it.
- PE transposes flip gathered blocks feature-major (8 blocks per PSUM
  bank, one DVE 2x copy per bank). Context projection cp = uT^T Ac^T runs
  in 512-col chunks, bias folded into the PSUM->SBUF copy. bu = u W^T runs
  per batch (windowed lhsT keeps outputs at partition 0) with rbeff added
  during the grouped PSUM->SBUF copy.
- Norms: per-batch matmul-with-ones into shared PSUM columns, then 4
  batched Ln/Exp activations for 1/|x| = exp(-0.5*ln(x^2+eps)).
- Per-batch softmax chain: dot matmul -> DVE row scale (ntinv) -> PE
  transpose -> Act exp (scale=ncinv, bias=pad mask) -> cs matmul into a
  shared column. Group reciprocal per 16 batches; output stage (o matmul,
  then a pure rsinv scale split across DVE/Act) runs one group behind,
  flushed to HBM in 8-batch DMAs.
"""

import numpy as np
from contextlib import ExitStack

# Problem constants (hardcoded per contract).
V, E, D = 100000, 128, 60
B, J, M, P = 512, 101, 50, 5120
NCORES = 8
BLOC = B // NCORES          # 64 batches per core
NTW = BLOC * J              # 6464 target rows per core
NU = BLOC * M               # 3200 context rows per core
NTWB = (NTW + 127) // 128   # 51 tight tw blocks
NUB = NU // 128             # 25 tight u blocks
NEG = -1.0e30
EPS2 = 1e-12

_CACHE = {}

_ACT_TABLE = "natural_log_exp_and_others"


def _patched_tables(orig_fn):
    def fn(arch):
        tabs = orig_fn(arch)
        return {
            name: (s if name == _ACT_TABLE else type(s)())
            for name, s in tabs.items()
        }
    return fn


def _build_program():
    import os
    NOPATCH = os.environ.get("K_NOPATCH") == "1"
    import concourse.bass as bass
    import concourse.tile as tile
    import concourse.bacc as bacc_mod
    from concourse import bacc, mybir

    f32 = mybir.dt.float32
    bf16 = mybir.dt.bfloat16
    i32 = mybir.dt.int32

    nc = bacc.Bacc(
        "TRN2",
        target_bir_lowering=False,
        debug=False,
        enable_asserts=False,
    )

    twe = nc.dram_tensor("twe", [V, D], bf16, kind="ExternalInput").ap()
    cemb = nc.dram_tensor("cemb", [V, E], bf16, kind="ExternalInput").ap()
    acw = nc.dram_tensor("acw", [E, D], bf16, kind="ExternalInput").ap()
    wrb = nc.dram_tensor("wrb", [E, E], bf16, kind="ExternalInput").ap()
    acb = nc.dram_tensor("acb", [D, 1], f32, kind="ExternalInput").ap()
    rbeff = nc.dram_tensor("rbeff", [1, E], f32, kind="ExternalInput").ap()
    eye = nc.dram_tensor("eye", [128, 128], bf16, kind="ExternalInput").ap()
    offt = nc.dram_tensor("offt", [128, NTWB], i32, kind="ExternalInput").ap()
    offc = nc.dram_tensor("offc", [128, NUB], i32, kind="ExternalInput").ap()
    maskT = nc.dram_tensor("maskT", [M, BLOC], f32, kind="ExternalInput").ap()
    out = nc.dram_tensor("out", [BLOC, J, E], f32, kind="ExternalOutput").ap()

    AF = mybir.ActivationFunctionType
    ALU = mybir.AluOpType

    with tile.TileContext(nc) as tc, ExitStack() as ctx:
        const = ctx.enter_context(tc.tile_pool(name="const", bufs=1))
        big = ctx.enter_context(tc.tile_pool(name="big", bufs=1))
        outp = ctx.enter_context(tc.tile_pool(name="outp", bufs=4))
        dotp = ctx.enter_context(tc.tile_pool(name="dotp", bufs=12))

        # --- constants (gather offsets first so the SWDGE stream starts early)
        offc_t = const.tile([128, NUB], i32)
        nc.sync.dma_start(out=offc_t[:], in_=offc[:, :])
        offt_t = const.tile([128, NTWB], i32)
        nc.sync.dma_start(out=offt_t[:], in_=offt[:, :])
        eye_t = const.tile([128, 128], bf16)
        nc.sync.dma_start(out=eye_t[:], in_=eye[:, :])
        acw_t = const.tile([E, D], bf16)
        nc.sync.dma_start(out=acw_t[:], in_=acw[:, :])
        wrb_t = const.tile([E, E], bf16)
        nc.sync.dma_start(out=wrb_t[:], in_=wrb[:, :])
        acb_t = const.tile([D, 1], f32)
        nc.sync.dma_start(out=acb_t[:], in_=acb[:, :])
        rb4_t = const.tile([M, 4 * E], f32)
        rb4_bcast = bass.AP(tensor=rbeff.tensor, offset=0,
                            ap=[[0, M], [0, 4], [1, E]])
        rb4_dst = rb4_t[:]
        nc.sync.dma_start(
            out=bass.AP(tensor=rb4_dst.tensor, offset=rb4_dst.offset,
                        ap=[rb4_dst.ap[0], [E, 4], [1, E]]),
            in_=rb4_bcast,
        )
        maskT_t = const.tile([M, BLOC], f32)
        nc.sync.dma_start(out=maskT_t[:], in_=maskT[:, :])
        ones_t = const.tile([128, 1], bf16)
        nc.vector.memset(ones_t[:], 1.0)
        eps_t = const.tile([128, 1], f32)
        nc.vector.memset(eps_t[:], EPS2)

        # --- big SBUF arrays ---
        twg = big.tile([128, NTWB, D], bf16)     # gathered tw rows, tight
        ug = big.tile([128, NUB, E], bf16)       # gathered c_emb rows, tight
        tpT_all = big.tile([D, NTWB * 128], bf16)  # feature-major tp
        uT_all = big.tile([E, NU], bf16)         # feature-major u
        cpT_all = big.tile([D, NU], bf16)        # context projection (+Ac_b)
        tp2_all = big.tile([D, NTWB * 128], bf16)
        cp2_all = big.tile([D, NU], bf16)
        bu_all = big.tile([M, BLOC * E], bf16)   # u @ (R Bc)^T + rbeff
        attnT = big.tile([M, BLOC * J], bf16)
        ntinv = big.tile([128, BLOC], f32)
        ncinv = big.tile([M, BLOC], f32)
        lnt_t = big.tile([128, BLOC], f32)
        lnc_t = big.tile([M, BLOC], f32)
        rs_all = big.tile([128, BLOC], f32)

        # --- gathers: 128 rows per indirect DMA (one desc per partition).
        # u blocks first (they feed the early u-side pipeline), then tw.
        for c in range(NUB):
            nc.gpsimd.indirect_dma_start(
                out=ug[:, c, :], out_offset=None, in_=cemb[:, :],
                in_offset=bass.IndirectOffsetOnAxis(
                    ap=offc_t[:, c : c + 1], axis=0
                ),
            )
        for c in range(NTWB):
            nc.gpsimd.indirect_dma_start(
                out=twg[:, c, :], out_offset=None, in_=twe[:, :],
                in_offset=bass.IndirectOffsetOnAxis(
                    ap=offt_t[:, c : c + 1], axis=0
                ),
            )

        if True:
            ps_tr = ctx.enter_context(
                tc.tile_pool(name="ps_tr", bufs=1, space="PSUM"))
            ps_sm = ctx.enter_context(
                tc.tile_pool(name="ps_sm", bufs=1, space="PSUM"))

            # nt columns and cs columns share one PSUM bank
            sm_ps = ps_sm.tile([128, 128], f32, tag="sm", space="PSUM")
            nt_ps = sm_ps[:, :BLOC]
            cs_ps = sm_ps[:, BLOC : 2 * BLOC]

            # ---- u side: transposes, projection, bu, context norms ----
            with tc.tile_pool(name="ps_pj", bufs=2, space="PSUM") as ps_pj, \
                 tc.tile_pool(name="ps_nc", bufs=1, space="PSUM") as ps_nc:
                for g in range((NUB + 7) // 8):
                    nblk = min(8, NUB - 8 * g)
                    floor_ms = (3000 + 1038 * (8 * g + nblk)) * 1e-6
                    with tc.tile_wait_until(floor_ms):
                        tr = ps_tr.tile([128, 1024], bf16, tag="tr",
                                        space="PSUM")
                        for i in range(nblk):
                            nc.tensor.transpose(
                                out=tr[:, 128 * i : 128 * (i + 1)],
                                in_=ug[:, 8 * g + i, :],
                                identity=eye_t[:, :],
                            )
                        nc.vector.tensor_copy(
                            out=uT_all[:, 1024 * g : 1024 * g + 128 * nblk],
                            in_=tr[:, : 128 * nblk],
                        )
                for c in range((NU + 511) // 512):
                    s = 512 * c
                    w = min(512, NU - s)
                    pj = ps_pj.tile([128, 512], f32, tag="pj", space="PSUM")
                    nc.tensor.matmul(
                        out=pj[:D, :w], lhsT=acw_t[:], rhs=uT_all[:, s : s + w],
                        start=True, stop=True,
                    )
                    if c % 2 == 0:
                        nc.vector.tensor_scalar_add(
                            out=cpT_all[:, s : s + w], in0=pj[:D, :w],
                            scalar1=acb_t[:],
                        )
                    else:
                        nc.scalar.activation(
                            out=cpT_all[:, s : s + w], in_=pj[:D, :w],
                            func=AF.Identity, bias=acb_t[:], scale=1.0,
                        )
                    nc.vector.tensor_mul(
                        out=cp2_all[:, s : s + w],
                        in0=cpT_all[:, s : s + w],
                        in1=cpT_all[:, s : s + w],
                    )
                nc_ps = ps_nc.tile([M, BLOC], f32, tag="ncn", space="PSUM")
                for b in range(BLOC):
                    nc.tensor.matmul(
                        out=nc_ps[:, b : b + 1],
                        lhsT=cp2_all[:, M * b : M * (b + 1)],
                        rhs=ones_t[:D, :],
                        start=True, stop=True,
                    )
                nc.scalar.activation(
                    out=lnc_t[:], in_=nc_ps[:, :], func=AF.Ln, bias=eps_t[:M]
                )
                nc.scalar.activation(
                    out=ncinv[:], in_=lnc_t[:], func=AF.Exp, scale=-0.5
                )
                for c in range(BLOC // 4):
                    bu_ps = ps_pj.tile([128, 512], f32, tag="pj", space="PSUM")
                    for i in range(4):
                        b = 4 * c + i
                        nc.tensor.matmul(
                            out=bu_ps[:M, 128 * i : 128 * (i + 1)],
                            lhsT=uT_all[:, M * b : M * b + M],
                            rhs=wrb_t[:],
                            start=True, stop=True,
                        )
                    nc.vector.scalar_tensor_tensor(
                        out=bu_all[:, 512 * c : 512 * (c + 1)],
                        in0=bu_ps[:M, :], scalar=1.0,
                        in1=rb4_t[:],
                        op0=ALU.mult, op1=ALU.add,
                    )

            # ---- tw side + attention, interleaved with the gather stream ----
            # One PSUM bank per in-flight batch: dot at byte 0 (f32 [101,50]),
            # dotT at byte 1024 (bf16 [50,128] via bitcast). 4 banks = depth 4.
            ps_ch = ctx.enter_context(
                tc.tile_pool(name="ps_ch", bufs=4, space="PSUM"))
            ps_o = ctx.enter_context(
                tc.tile_pool(name="ps_o", bufs=2, space="PSUM"))
            GRP = 16
            pendB = []
            obuf = None

            def stageA(b):
                s = J * b
                ch = ps_ch.tile([128, 512], f32, tag="ch", space="PSUM")
                dot_ps = ch[:J, :M]
                nc.tensor.matmul(
                    out=dot_ps,
                    lhsT=tpT_all[:, s : s + J],
                    rhs=cpT_all[:, M * b : M * b + M],
                    start=True, stop=True,
                )
                dotn = dotp.tile([J, M], bf16, tag="dotn")
                nc.vector.tensor_scalar_mul(
                    dotn[:], dot_ps, ntinv[:J, b : b + 1]
                )
                dT_ps = ch[:M, 256:320].bitcast(bf16)
                nc.tensor.transpose(
                    out=dT_ps[:, :J], in_=dotn[:], identity=eye_t[:J, :J]
                )
                nc.scalar.activation(
                    out=attnT[:, s : s + J],
                    in_=dT_ps[:, :J],
                    func=AF.Exp,
                    bias=maskT_t[:, b : b + 1],
                    scale=ncinv[:, b : b + 1],
                )

            def stageCS(b):
                s = J * b
                nc.tensor.matmul(
                    out=cs_ps[:J, b : b + 1],
                    lhsT=attnT[:, s : s + J],
                    rhs=ones_t[:M, :],
                    start=True, stop=True,
                )

            def stageB(b, buf):
                s = J * b
                o_ps = ps_o.tile([J, E], f32, tag="o", space="PSUM")
                nc.tensor.matmul(
                    out=o_ps[:, :],
                    lhsT=attnT[:, s : s + J],
                    rhs=bu_all[:, E * b : E * (b + 1)],
                    start=True, stop=True,
                )
                k = b % 8
                if True:
                    nc.vector.tensor_scalar_mul(
                        buf[:, E * k : E * (k + 1)], o_ps[:, :],
                        rs_all[:J, b : b + 1],
                    )
                else:
                    nc.scalar.activation(
                        out=buf[:, E * k : E * (k + 1)], in_=o_ps[:, :],
                        func=AF.Identity, scale=rs_all[:J, b : b + 1],
                    )

            def flush(b0, n, buf):
                k0 = (b0 % 8) * E
                dst = bass.AP(
                    tensor=out.tensor,
                    offset=b0 * J * E,
                    ap=[[E, J], [J * E, n], [1, E]],
                )
                nc.sync.dma_start(out=dst, in_=buf[:, k0 : k0 + n * E])

            fbase = [0]

            def runB():
                nonlocal obuf
                b = pendB.pop(0)
                if obuf is None:
                    obuf = outp.tile([J, 8 * E], f32, tag="ob")
                    fbase[0] = b
                stageB(b, obuf)
                n = b - fbase[0] + 1
                if n == 8 or b == BLOC - 1 or (
                        b >= BLOC - 8 and (b + 1) % 4 == 0) or (
                        b >= BLOC - 8 and (b + 1) % 2 == 0):
                    flush(fbase[0], n, obuf)
                    obuf = None

            def window(lo, hi):
                # ntinv for batches [lo, hi), then their softmax chain
                n = hi - lo
                nc.scalar.activation(
                    out=lnt_t[:J, lo:hi], in_=nt_ps[:J, lo:hi],
                    func=AF.Ln, bias=eps_t[:J],
                )
                nc.scalar.activation(
                    out=ntinv[:J, lo:hi], in_=lnt_t[:J, lo:hi],
                    func=AF.Exp, scale=-0.5,
                )
                for b in range(lo, hi):
                    stageA(b)
                    if pendB:
                        runB()
                for b in range(lo, hi):
                    stageCS(b)
                nc.vector.reciprocal(
                    out=rs_all[:J, lo:hi], in_=cs_ps[:J, lo:hi],
                )
                pendB.extend(range(lo, hi))

            done_b = 0   # batches whose nt-norm matmul has been emitted
            blk0 = 0
            GSIZES = [8, 8, 8, 8, 8, 4, 4, 1, 1, 1]
            assert sum(GSIZES) == NTWB
            for nblk in GSIZES:
                floor_ms = (3000 + 1038 * (NUB + blk0 + nblk)) * 1e-6
                with tc.tile_wait_until(floor_ms):
                    tr = ps_tr.tile([128, 1024], bf16, tag="tr", space="PSUM")
                    for i in range(nblk):
                        nc.tensor.transpose(
                            out=tr[:D, 128 * i : 128 * (i + 1)],
                            in_=twg[:, blk0 + i, :],
                            identity=eye_t[:, :],
                        )
                    s2 = 128 * blk0
                    nc.vector.tensor_copy(
                        out=tpT_all[:, s2 : s2 + 128 * nblk],
                        in_=tr[:D, : 128 * nblk],
                    )
                    nc.vector.tensor_mul(
                        out=tp2_all[:, s2 : s2 + 128 * nblk],
                        in0=tpT_all[:, s2 : s2 + 128 * nblk],
                        in1=tpT_all[:, s2 : s2 + 128 * nblk],
                    )
                    blk0 += nblk
                    nb = min(BLOC, (128 * blk0) // J)
                    for b in range(done_b, nb):
                        nc.tensor.matmul(
                            out=nt_ps[:J, b : b + 1],
                            lhsT=tp2_all[:, J * b : J * (b + 1)],
                            rhs=ones_t[:D, :],
                            start=True, stop=True,
                        )
                    if nb > done_b:
                        window(done_b, nb)
                    done_b = nb
            while pendB:
                runB()

    if NOPATCH:
        nc.compile()
    else:
        orig = bacc_mod.get_activation_tables
        bacc_mod.get_activation_tables = _patched_tables(orig)
        try:
            nc.compile()
        finally:
            bacc_mod.get_activation_tables = orig
    return nc


def _get_program():
    if "nc" not in _CACHE:
        _CACHE["nc"] = _build_program()
    return _CACHE["nc"]


def _prep_inputs(batch_titems, batch_citems, batch_pad_ids, t_emb, c_emb,
                 Ac_w, Ac_b, At_w, At_b, Bc_w, Bc_b, R_w, R_b):
    import ml_dtypes
    bf16 = ml_dtypes.bfloat16

    f = lambda x: np.ascontiguousarray(np.asarray(x, dtype=np.float32))
    t32 = np.asarray(t_emb, np.float32)
    c32 = np.asarray(c_emb, np.float32)
    At = np.asarray(At_w, np.float32)
    Ac = np.asarray(Ac_w, np.float32)
    Bc = np.asarray(Bc_w, np.float32)
    R = np.asarray(R_w, np.float32)

    # tw rows = t_emb @ At^T + At_b (bias folded into the table)
    twe = np.ascontiguousarray(
        (t32 @ At.T + np.asarray(At_b, np.float32)).astype(bf16))
    cemb_b = np.ascontiguousarray(c32.astype(bf16))

    tit = np.asarray(batch_titems).astype(np.int32)
    cit = np.asarray(batch_citems).astype(np.int32)
    pad = np.asarray(batch_pad_ids).astype(np.int64)

    mask = np.zeros((B, M), np.float32)
    mask[pad[0], pad[1]] = NEG

    acw = np.ascontiguousarray(Ac.T.astype(bf16))
    wrb = np.ascontiguousarray((R @ Bc).T.astype(bf16))
    acb = f(np.asarray(Ac_b).reshape(D, 1))
    rbeff = f(
        (np.asarray(R_b, np.float32) + R @ np.asarray(Bc_b, np.float32)
         ).reshape(1, E)
    )
    eye = np.eye(128, dtype=np.float32).astype(bf16)

    in_maps = []
    for c in range(NCORES):
        s = c * BLOC
        # tight tw offsets: item i = (b*J + j) at partition i%128, block i//128
        tflat = tit[s : s + BLOC].reshape(-1)  # [6464]
        tpad = np.zeros(NTWB * 128, np.int32)
        tpad[:NTW] = tflat
        offt = np.ascontiguousarray(tpad.reshape(NTWB, 128).T)

        cflat = cit[s : s + BLOC].reshape(-1)  # [3200]
        offc = np.ascontiguousarray(cflat.reshape(NUB, 128).T)

        maskTc = np.ascontiguousarray(mask[s : s + BLOC].T)  # [50,64]

        in_maps.append(
            {
                "twe": twe,
                "cemb": cemb_b,
                "acw": acw,
                "wrb": wrb,
                "acb": acb,
                "rbeff": rbeff,
                "eye": eye,
                "offt": offt,
                "offc": offc,
                "maskT": maskTc,
            }
        )
    return in_maps


def run_sharded(in_maps, **kwargs):
    from concourse.bass_utils import run_bass_kernel_spmd

    nc = _get_program()
    res = run_bass_kernel_spmd(nc, in_maps, core_ids=list(range(NCORES)), **kwargs)
    outs = [res.results[c]["out"] for c in range(NCORES)]
    full = np.concatenate(outs, axis=0)
    return full, res


def kernel(**inputs):
    in_maps = _prep_inputs(**inputs)
    full, _ = run_sharded(in_maps)
    return full.astype(np.float32)


# revision 56
# speedup vs baseline: 1.0288x; 1.0020x over previous
"""AttentiveItemToVec Trainium2 kernel (batched bf16, host-folded weights).

Full-input contract: kernel(**inputs) takes the unsharded numpy inputs and
returns the full [512, 101, 128] float32 output. Internally shards the batch
across 8 NeuronCores (64 batches each), runs a Bass/Tile kernel per core via
run_bass_kernel_spmd, and concatenates the per-core outputs.

Host prep folds weight-space linear maps (data-independent):
  tw = t_emb @ At_w^T + At_b  [V, 60] -> the gather IS the target projection
  W  = (R_w Bc_w)^T                   -> R folded into Bc (one output matmul)
  rbeff = R_b + R_w @ Bc_b            -> folded into the bu rows on device

Device (per core, 64 batches):
- Gathers use indirect DMA at its max of 128 rows/instruction (this
  runtime generates one descriptor per partition): 51 tight tw blocks +
  25 tight raw-u blocks = 76 instructions saturating the GpSimd queue;
  everything else is sized to hide under it.
- PE transposes flip gathered blocks feature-major (8 blocks per PSUM
  bank, one DVE 2x copy per bank). Context projection cp = uT^T Ac^T runs
  in 512-col chunks, bias folded into the PSUM->SBUF copy. bu = u W^T runs
  per batch (windowed lhsT keeps outputs at partition 0) with rbeff added
  during the grouped PSUM->SBUF copy.
- Norms: per-batch matmul-with-ones into shared PSUM columns, then 4
  batched Ln/Exp activations for 1/|x| = exp(-0.5*ln(x^2+eps)).
- Per-batch softmax chain: dot matmul -> DVE row scale (ntinv) -> PE
  transpose -> Act exp (scale=ncinv, bias=pad mask) -> cs matmul into a
  shared column. Group reciprocal per 16 batches; output stage (o matmul,
  then a pure rsinv scale split across DVE/Act) runs one group behind,
  flushed to HBM in 8-batch DMAs.
"""

import numpy as np
from contextlib import ExitStack

# Problem constants (hardcoded per contract).
V, E, D = 100000, 128, 60
B, J, M, P = 512, 101, 50, 5120
NCORES = 8
BLOC = B // NCORES          # 64 batches per core
NTW = BLOC * J              # 6464 target rows per core
NU = BLOC * M               # 3200 context rows per core
NTWB = (NTW + 127) // 128   # 51 tight tw blocks
NUB = NU // 128             # 25 tight u blocks
NEG = -1.0e30
EPS2 = 1e-12

_CACHE = {}

_ACT_TABLE = "natural_log_exp_and_others"


def _patched_tables(orig_fn):
    def fn(arch):
        tabs = orig_fn(arch)
        return {
            name: (s if name == _ACT_TABLE else type(s)())
            for name, s in tabs.items()
        }
    return fn


def _build_program():
    import os
    NOPATCH = os.environ.get("K_NOPATCH") == "1"
    import concourse.bass as bass
    import concourse.tile as tile
    import concourse.bacc as bacc_mod
    from concourse import bacc, mybir

    f32 = mybir.dt.float32
    bf16 = mybir.dt.bfloat16
    i32 = mybir.dt.int32

    nc = bacc.Bacc(
        "TRN2",
        target_bir_lowering=False,
        debug=False,
        enable_asserts=False,
    )

    twe = nc.dram_tensor("twe", [V, D], bf16, kind="ExternalInput").ap()
    cemb = nc.dram_tensor("cemb", [V, E], bf16, kind="ExternalInput").ap()
    acw = nc.dram_tensor("acw", [E, D], bf16, kind="ExternalInput").ap()
    wrb = nc.dram_tensor("wrb", [E, E], bf16, kind="ExternalInput").ap()
    acb = nc.dram_tensor("acb", [D, 1], f32, kind="ExternalInput").ap()
    rbeff = nc.dram_tensor("rbeff", [1, E], f32, kind="ExternalInput").ap()
    eye = nc.dram_tensor("eye", [128, 128], bf16, kind="ExternalInput").ap()
    offt = nc.dram_tensor("offt", [128, NTWB], i32, kind="ExternalInput").ap()
    offc = nc.dram_tensor("offc", [128, NUB], i32, kind="ExternalInput").ap()
    maskT = nc.dram_tensor("maskT", [M, BLOC], f32, kind="ExternalInput").ap()
    out = nc.dram_tensor("out", [BLOC, J, E], f32, kind="ExternalOutput").ap()

    AF = mybir.ActivationFunctionType
    ALU = mybir.AluOpType

    with tile.TileContext(nc) as tc, ExitStack() as ctx:
        const = ctx.enter_context(tc.tile_pool(name="const", bufs=1))
        big = ctx.enter_context(tc.tile_pool(name="big", bufs=1))
        outp = ctx.enter_context(tc.tile_pool(name="outp", bufs=4))
        dotp = ctx.enter_context(tc.tile_pool(name="dotp", bufs=12))

        # --- constants (gather offsets first so the SWDGE stream starts early)
        offc_t = const.tile([128, NUB], i32)
        nc.sync.dma_start(out=offc_t[:], in_=offc[:, :])
        offt_t = const.tile([128, NTWB], i32)
        nc.sync.dma_start(out=offt_t[:], in_=offt[:, :])
        eye_t = const.tile([128, 128], bf16)
        nc.sync.dma_start(out=eye_t[:], in_=eye[:, :])
        acw_t = const.tile([E, D], bf16)
        nc.sync.dma_start(out=acw_t[:], in_=acw[:, :])
        wrb_t = const.tile([E, E], bf16)
        nc.sync.dma_start(out=wrb_t[:], in_=wrb[:, :])
        acb_t = const.tile([D, 1], f32)
        nc.sync.dma_start(out=acb_t[:], in_=acb[:, :])
        rb4_t = const.tile([M, 4 * E], f32)
        rb4_bcast = bass.AP(tensor=rbeff.tensor, offset=0,
                            ap=[[0, M], [0, 4], [1, E]])
        rb4_dst = rb4_t[:]
        nc.sync.dma_start(
            out=bass.AP(tensor=rb4_dst.tensor, offset=rb4_dst.offset,
                        ap=[rb4_dst.ap[0], [E, 4], [1, E]]),
            in_=rb4_bcast,
        )
        maskT_t = const.tile([M, BLOC], f32)
        nc.sync.dma_start(out=maskT_t[:], in_=maskT[:, :])
        ones_t = const.tile([128, 1], bf16)
        nc.vector.memset(ones_t[:], 1.0)
        eps_t = const.tile([128, 1], f32)
        nc.vector.memset(eps_t[:], EPS2)

        # --- big SBUF arrays ---
        twg = big.tile([128, NTWB, D], bf16)     # gathered tw rows, tight
        ug = big.tile([128, NUB, E], bf16)       # gathered c_emb rows, tight
        tpT_all = big.tile([D, NTWB * 128], bf16)  # feature-major tp
        uT_all = big.tile([E, NU], bf16)         # feature-major u
        cpT_all = big.tile([D, NU], bf16)        # context projection (+Ac_b)
        tp2_all = big.tile([D, NTWB * 128], bf16)
        cp2_all = big.tile([D, NU], bf16)
        bu_all = big.tile([M, BLOC * E], bf16)   # u @ (R Bc)^T + rbeff
        attnT = big.tile([M, BLOC * J], bf16)
        ntinv = big.tile([128, BLOC], f32)
        ncinv = big.tile([M, BLOC], f32)
        lnt_t = big.tile([128, BLOC], f32)
        lnc_t = big.tile([M, BLOC], f32)
        rs_all = big.tile([128, BLOC], f32)

        # --- gathers: 128 rows per indirect DMA (one desc per partition).
        # u blocks first (they feed the early u-side pipeline), then tw.
        for c in range(NUB):
            nc.gpsimd.indirect_dma_start(
                out=ug[:, c, :], out_offset=None, in_=cemb[:, :],
                in_offset=bass.IndirectOffsetOnAxis(
                    ap=offc_t[:, c : c + 1], axis=0
                ),
            )
        for c in range(NTWB):
            nc.gpsimd.indirect_dma_start(
                out=twg[:, c, :], out_offset=None, in_=twe[:, :],
                in_offset=bass.IndirectOffsetOnAxis(
                    ap=offt_t[:, c : c + 1], axis=0
                ),
            )

        if True:
            ps_tr = ctx.enter_context(
                tc.tile_pool(name="ps_tr", bufs=1, space="PSUM"))
            ps_sm = ctx.enter_context(
                tc.tile_pool(name="ps_sm", bufs=1, space="PSUM"))

            # nt columns and cs columns share one PSUM bank
            sm_ps = ps_sm.tile([128, 128], f32, tag="sm", space="PSUM")
            nt_ps = sm_ps[:, :BLOC]
            cs_ps = sm_ps[:, BLOC : 2 * BLOC]

            # ---- u side: transposes, projection, bu, context norms ----
            with tc.tile_pool(name="ps_pj", bufs=2, space="PSUM") as ps_pj, \
                 tc.tile_pool(name="ps_nc", bufs=1, space="PSUM") as ps_nc:
                for g in range((NUB + 7) // 8):
                    nblk = min(8, NUB - 8 * g)
                    floor_ms = (4600 + 1038 * (8 * g + nblk)) * 1e-6
                    with tc.tile_wait_until(floor_ms):
                        tr = ps_tr.tile([128, 1024], bf16, tag="tr",
                                        space="PSUM")
                        for i in range(nblk):
                            nc.tensor.transpose(
                                out=tr[:, 128 * i : 128 * (i + 1)],
                                in_=ug[:, 8 * g + i, :],
                                identity=eye_t[:, :],
                            )
                        nc.vector.tensor_copy(
                            out=uT_all[:, 1024 * g : 1024 * g + 128 * nblk],
                            in_=tr[:, : 128 * nblk],
                        )
                for c in range((NU + 511) // 512):
                    s = 512 * c
                    w = min(512, NU - s)
                    pj = ps_pj.tile([128, 512], f32, tag="pj", space="PSUM")
                    nc.tensor.matmul(
                        out=pj[:D, :w], lhsT=acw_t[:], rhs=uT_all[:, s : s + w],
                        start=True, stop=True,
                    )
                    if c % 2 == 0:
                        nc.vector.tensor_scalar_add(
                            out=cpT_all[:, s : s + w], in0=pj[:D, :w],
                            scalar1=acb_t[:],
                        )
                    else:
                        nc.scalar.activation(
                            out=cpT_all[:, s : s + w], in_=pj[:D, :w],
                            func=AF.Identity, bias=acb_t[:], scale=1.0,
                        )
                    nc.vector.tensor_mul(
                        out=cp2_all[:, s : s + w],
                        in0=cpT_all[:, s : s + w],
                        in1=cpT_all[:, s : s + w],
                    )
                nc_ps = ps_nc.tile([M, BLOC], f32, tag="ncn", space="PSUM")
                for b in range(BLOC):
                    nc.tensor.matmul(
                        out=nc_ps[:, b : b + 1],
                        lhsT=cp2_all[:, M * b : M * (b + 1)],
                        rhs=ones_t[:D, :],
                        start=True, stop=True,
                    )
                nc.scalar.activation(
                    out=lnc_t[:], in_=nc_ps[:, :], func=AF.Ln, bias=eps_t[:M]
                )
                nc.scalar.activation(
                    out=ncinv[:], in_=lnc_t[:], func=AF.Exp, scale=-0.5
                )
                for c in range(BLOC // 4):
                    bu_ps = ps_pj.tile([128, 512], f32, tag="pj", space="PSUM")
                    for i in range(4):
                        b = 4 * c + i
                        nc.tensor.matmul(
                            out=bu_ps[:M, 128 * i : 128 * (i + 1)],
                            lhsT=uT_all[:, M * b : M * b + M],
                            rhs=wrb_t[:],
                            start=True, stop=True,
                        )
                    nc.vector.scalar_tensor_tensor(
                        out=bu_all[:, 512 * c : 512 * (c + 1)],
                        in0=bu_ps[:M, :], scalar=1.0,
                        in1=rb4_t[:],
                        op0=ALU.mult, op1=ALU.add,
                    )

            # ---- tw side + attention, interleaved with the gather stream ----
            # One PSUM bank per in-flight batch: dot at byte 0 (f32 [101,50]),
            # dotT at byte 1024 (bf16 [50,128] via bitcast). 4 banks = depth 4.
            ps_ch = ctx.enter_context(
                tc.tile_pool(name="ps_ch", bufs=4, space="PSUM"))
            ps_o = ctx.enter_context(
                tc.tile_pool(name="ps_o", bufs=2, space="PSUM"))
            GRP = 16
            pendB = []
            obuf = None

            def stageA(b):
                s = J * b
                ch = ps_ch.tile([128, 512], f32, tag="ch", space="PSUM")
                dot_ps = ch[:J, :M]
                nc.tensor.matmul(
                    out=dot_ps,
                    lhsT=tpT_all[:, s : s + J],
                    rhs=cpT_all[:, M * b : M * b + M],
                    start=True, stop=True,
                )
                dotn = dotp.tile([J, M], bf16, tag="dotn")
                nc.vector.tensor_scalar_mul(
                    dotn[:], dot_ps, ntinv[:J, b : b + 1]
                )
                dT_ps = ch[:M, 256:320].bitcast(bf16)
                nc.tensor.transpose(
                    out=dT_ps[:, :J], in_=dotn[:], identity=eye_t[:J, :J]
                )
                nc.scalar.activation(
                    out=attnT[:, s : s + J],
                    in_=dT_ps[:, :J],
                    func=AF.Exp,
                    bias=maskT_t[:, b : b + 1],
                    scale=ncinv[:, b : b + 1],
                )

            def stageCS(b):
                s = J * b
                nc.tensor.matmul(
                    out=cs_ps[:J, b : b + 1],
                    lhsT=attnT[:, s : s + J],
                    rhs=ones_t[:M, :],
                    start=True, stop=True,
                )

            def stageB(b, buf):
                s = J * b
                o_ps = ps_o.tile([J, E], f32, tag="o", space="PSUM")
                nc.tensor.matmul(
                    out=o_ps[:, :],
                    lhsT=attnT[:, s : s + J],
                    rhs=bu_all[:, E * b : E * (b + 1)],
                    start=True, stop=True,
                )
                k = b % 8
                if True:
                    nc.vector.tensor_scalar_mul(
                        buf[:, E * k : E * (k + 1)], o_ps[:, :],
                        rs_all[:J, b : b + 1],
                    )
                else:
                    nc.scalar.activation(
                        out=buf[:, E * k : E * (k + 1)], in_=o_ps[:, :],
                        func=AF.Identity, scale=rs_all[:J, b : b + 1],
                    )

            def flush(b0, n, buf):
                k0 = (b0 % 8) * E
                dst = bass.AP(
                    tensor=out.tensor,
                    offset=b0 * J * E,
                    ap=[[E, J], [J * E, n], [1, E]],
                )
                nc.sync.dma_start(out=dst, in_=buf[:, k0 : k0 + n * E])

            fbase = [0]

            def runB():
                nonlocal obuf
                b = pendB.pop(0)
                if obuf is None:
                    obuf = outp.tile([J, 8 * E], f32, tag="ob")
                    fbase[0] = b
                stageB(b, obuf)
                n = b - fbase[0] + 1
                if n == 8 or b == BLOC - 1 or (
                        b >= BLOC - 8 and (b + 1) % 4 == 0) or (
                        b >= BLOC - 8 and (b + 1) % 2 == 0) or b >= BLOC - 2:
                    flush(fbase[0], n, obuf)
                    obuf = None

            def window(lo, hi):
                # ntinv for batches [lo, hi), then their softmax chain
                n = hi - lo
                nc.scalar.activation(
                    out=lnt_t[:J, lo:hi], in_=nt_ps[:J, lo:hi],
                    func=AF.Ln, bias=eps_t[:J],
                )
                nc.scalar.activation(
                    out=ntinv[:J, lo:hi], in_=lnt_t[:J, lo:hi],
                    func=AF.Exp, scale=-0.5,
                )
                for b in range(lo, hi):
                    stageA(b)
                    if pendB:
                        runB()
                for b in range(lo, hi):
                    stageCS(b)
                nc.vector.reciprocal(
                    out=rs_all[:J, lo:hi], in_=cs_ps[:J, lo:hi],
                )
                pendB.extend(range(lo, hi))

            done_b = 0   # batches whose nt-norm matmul has been emitted
            blk0 = 0
            GSIZES = [8, 8, 8, 8, 8, 4, 4, 1, 1, 1]
            assert sum(GSIZES) == NTWB
            for nblk in GSIZES:
                floor_ms = (4600 + 1038 * (NUB + blk0 + nblk)) * 1e-6
                with tc.tile_wait_until(floor_ms):
                    tr = ps_tr.tile([128, 1024], bf16, tag="tr", space="PSUM")
                    for i in range(nblk):
                        nc.tensor.transpose(
                            out=tr[:D, 128 * i : 128 * (i + 1)],
                            in_=twg[:, blk0 + i, :],
                            identity=eye_t[:, :],
                        )
                    s2 = 128 * blk0
                    nc.vector.tensor_copy(
                        out=tpT_all[:, s2 : s2 + 128 * nblk],
                        in_=tr[:D, : 128 * nblk],
                    )
                    nc.vector.tensor_mul(
                        out=tp2_all[:, s2 : s2 + 128 * nblk],
                        in0=tpT_all[:, s2 : s2 + 128 * nblk],
                        in1=tpT_all[:, s2 : s2 + 128 * nblk],
                    )
                    blk0 += nblk
                    nb = min(BLOC, (128 * blk0) // J)
                    for b in range(done_b, nb):
                        nc.tensor.matmul(
                            out=nt_ps[:J, b : b + 1],
                            lhsT=tp2_all[:, J * b : J * (b + 1)],
                            rhs=ones_t[:D, :],
                            start=True, stop=True,
                        )
                    if nb > done_b:
                        window(done_b, nb)
                    done_b = nb
            while pendB:
                runB()

    if NOPATCH:
        nc.compile()
    else:
        orig = bacc_mod.get_activation_tables
        bacc_mod.get_activation_tables = _patched_tables(orig)
        try:
            nc.compile()
        finally:
            bacc_mod.get_activation_tables = orig
    return nc


def _get_program():
    if "nc" not in _CACHE:
        _CACHE["nc"] = _build_program()
    return _CACHE["nc"]


def _prep_inputs(batch_titems, batch_citems, batch_pad_ids, t_emb, c_emb,
                 Ac_w, Ac_b, At_w, At_b, Bc_w, Bc_b, R_w, R_b):
    import ml_dtypes
    bf16 = ml_dtypes.bfloat16

    f = lambda x: np.ascontiguousarray(np.asarray(x, dtype=np.float32))
    t32 = np.asarray(t_emb, np.float32)
    c32 = np.asarray(c_emb, np.float32)
    At = np.asarray(At_w, np.float32)
    Ac = np.asarray(Ac_w, np.float32)
    Bc = np.asarray(Bc_w, np.float32)
    R = np.asarray(R_w, np.float32)

    # tw rows = t_emb @ At^T + At_b (bias folded into the table)
    twe = np.ascontiguousarray(
        (t32 @ At.T + np.asarray(At_b, np.float32)).astype(bf16))
    cemb_b = np.ascontiguousarray(c32.astype(bf16))

    tit = np.asarray(batch_titems).astype(np.int32)
    cit = np.asarray(batch_citems).astype(np.int32)
    pad = np.asarray(batch_pad_ids).astype(np.int64)

    mask = np.zeros((B, M), np.float32)
    mask[pad[0], pad[1]] = NEG

    acw = np.ascontiguousarray(Ac.T.astype(bf16))
    wrb = np.ascontiguousarray((R @ Bc).T.astype(bf16))
    acb = f(np.asarray(Ac_b).reshape(D, 1))
    rbeff = f(
        (np.asarray(R_b, np.float32) + R @ np.asarray(Bc_b, np.float32)
         ).reshape(1, E)
    )
    eye = np.eye(128, dtype=np.float32).astype(bf16)

    in_maps = []
    for c in range(NCORES):
        s = c * BLOC
        # tight tw offsets: item i = (b*J + j) at partition i%128, block i//128
        tflat = tit[s : s + BLOC].reshape(-1)  # [6464]
        tpad = np.zeros(NTWB * 128, np.int32)
        tpad[:NTW] = tflat
        offt = np.ascontiguousarray(tpad.reshape(NTWB, 128).T)

        cflat = cit[s : s + BLOC].reshape(-1)  # [3200]
        offc = np.ascontiguousarray(cflat.reshape(NUB, 128).T)

        maskTc = np.ascontiguousarray(mask[s : s + BLOC].T)  # [50,64]

        in_maps.append(
            {
                "twe": twe,
                "cemb": cemb_b,
                "acw": acw,
                "wrb": wrb,
                "acb": acb,
                "rbeff": rbeff,
                "eye": eye,
                "offt": offt,
                "offc": offc,
                "maskT": maskTc,
            }
        )
    return in_maps


def run_sharded(in_maps, **kwargs):
    from concourse.bass_utils import run_bass_kernel_spmd

    nc = _get_program()
    res = run_bass_kernel_spmd(nc, in_maps, core_ids=list(range(NCORES)), **kwargs)
    outs = [res.results[c]["out"] for c in range(NCORES)]
    full = np.concatenate(outs, axis=0)
    return full, res


def kernel(**inputs):
    in_maps = _prep_inputs(**inputs)
    full, _ = run_sharded(in_maps)
    return full.astype(np.float32)


# revision 59
# speedup vs baseline: 1.0393x; 1.0102x over previous
"""AttentiveItemToVec Trainium2 kernel (batched bf16, host-folded weights).

Full-input contract: kernel(**inputs) takes the unsharded numpy inputs and
returns the full [512, 101, 128] float32 output. Internally shards the batch
across 8 NeuronCores (64 batches each), runs a Bass/Tile kernel per core via
run_bass_kernel_spmd, and concatenates the per-core outputs.

Host prep folds weight-space linear maps (data-independent):
  tw = t_emb @ At_w^T + At_b  [V, 60] -> the gather IS the target projection
  W  = (R_w Bc_w)^T                   -> R folded into Bc (one output matmul)
  rbeff = R_b + R_w @ Bc_b            -> folded into the bu rows on device

Device (per core, 64 batches):
- Gathers use indirect DMA at its max of 128 rows/instruction (this
  runtime generates one descriptor per partition): 51 tight tw blocks +
  25 tight raw-u blocks = 76 instructions saturating the GpSimd queue;
  everything else is sized to hide under it.
- PE transposes flip gathered blocks feature-major (8 blocks per PSUM
  bank, one DVE 2x copy per bank). Context projection cp = uT^T Ac^T runs
  in 512-col chunks, bias folded into the PSUM->SBUF copy. bu = u W^T runs
  per batch (windowed lhsT keeps outputs at partition 0) with rbeff added
  during the grouped PSUM->SBUF copy.
- Norms: per-batch matmul-with-ones into shared PSUM columns, then 4
  batched Ln/Exp activations for 1/|x| = exp(-0.5*ln(x^2+eps)).
- Per-batch softmax chain: dot matmul -> DVE row scale (ntinv) -> PE
  transpose -> Act exp (scale=ncinv, bias=pad mask) -> cs matmul into a
  shared column. Group reciprocal per 16 batches; output stage (o matmul,
  then a pure rsinv scale split across DVE/Act) runs one group behind,
  flushed to HBM in 8-batch DMAs.
"""

import numpy as np
from contextlib import ExitStack

# Problem constants (hardcoded per contract).
V, E, D = 100000, 128, 60
B, J, M, P = 512, 101, 50, 5120
NCORES = 8
BLOC = B // NCORES          # 64 batches per core
NTW = BLOC * J              # 6464 target rows per core
NU = BLOC * M               # 3200 context rows per core
NTWB = (NTW + 127) // 128   # 51 tight tw blocks
NUB = NU // 128             # 25 tight u blocks
NEG = -1.0e30
EPS2 = 1e-12

_CACHE = {}

_ACT_TABLE = "natural_log_exp_and_others"


def _patched_tables(orig_fn):
    def fn(arch):
        tabs = orig_fn(arch)
        return {
            name: (s if name == _ACT_TABLE else type(s)())
            for name, s in tabs.items()
        }
    return fn


def _build_program():
    import os
    NOPATCH = os.environ.get("K_NOPATCH") == "1"
    import concourse.bass as bass
    import concourse.tile as tile
    import concourse.bacc as bacc_mod
    from concourse import bacc, mybir

    f32 = mybir.dt.float32
    bf16 = mybir.dt.bfloat16
    i32 = mybir.dt.int32

    nc = bacc.Bacc(
        "TRN2",
        target_bir_lowering=False,
        debug=False,
        enable_asserts=False,
    )

    twe = nc.dram_tensor("twe", [V, D], bf16, kind="ExternalInput").ap()
    cemb = nc.dram_tensor("cemb", [V, E], bf16, kind="ExternalInput").ap()
    acw = nc.dram_tensor("acw", [E, D], bf16, kind="ExternalInput").ap()
    wrb = nc.dram_tensor("wrb", [E, E], bf16, kind="ExternalInput").ap()
    acb = nc.dram_tensor("acb", [D, 1], f32, kind="ExternalInput").ap()
    rbeff = nc.dram_tensor("rbeff", [1, E], f32, kind="ExternalInput").ap()
    eye = nc.dram_tensor("eye", [128, 128], bf16, kind="ExternalInput").ap()
    offt = nc.dram_tensor("offt", [128, NTWB], i32, kind="ExternalInput").ap()
    offc = nc.dram_tensor("offc", [128, NUB], i32, kind="ExternalInput").ap()
    maskT = nc.dram_tensor("maskT", [M, BLOC], f32, kind="ExternalInput").ap()
    out = nc.dram_tensor("out", [BLOC, J, E], f32, kind="ExternalOutput").ap()

    AF = mybir.ActivationFunctionType
    ALU = mybir.AluOpType

    with tile.TileContext(nc) as tc, ExitStack() as ctx:
        const = ctx.enter_context(tc.tile_pool(name="const", bufs=1))
        big = ctx.enter_context(tc.tile_pool(name="big", bufs=1))
        outp = ctx.enter_context(tc.tile_pool(name="outp", bufs=4))
        dotp = ctx.enter_context(tc.tile_pool(name="dotp", bufs=12))

        # --- constants (gather offsets first so the SWDGE stream starts early)
        offc_t = const.tile([128, NUB], i32)
        nc.sync.dma_start(out=offc_t[:], in_=offc[:, :])
        offt_t = const.tile([128, NTWB], i32)
        nc.sync.dma_start(out=offt_t[:], in_=offt[:, :])
        eye_t = const.tile([128, 128], bf16)
        nc.sync.dma_start(out=eye_t[:], in_=eye[:, :])
        acw_t = const.tile([E, D], bf16)
        nc.sync.dma_start(out=acw_t[:], in_=acw[:, :])
        wrb_t = const.tile([E, E], bf16)
        nc.sync.dma_start(out=wrb_t[:], in_=wrb[:, :])
        acb_t = const.tile([D, 1], f32)
        nc.sync.dma_start(out=acb_t[:], in_=acb[:, :])
        rb4_t = const.tile([M, 4 * E], f32)
        rb4_bcast = bass.AP(tensor=rbeff.tensor, offset=0,
                            ap=[[0, M], [0, 4], [1, E]])
        rb4_dst = rb4_t[:]
        nc.sync.dma_start(
            out=bass.AP(tensor=rb4_dst.tensor, offset=rb4_dst.offset,
                        ap=[rb4_dst.ap[0], [E, 4], [1, E]]),
            in_=rb4_bcast,
        )
        maskT_t = const.tile([M, BLOC], f32)
        nc.sync.dma_start(out=maskT_t[:], in_=maskT[:, :])
        ones_t = const.tile([128, 1], bf16)
        nc.vector.memset(ones_t[:], 1.0)
        eps_t = const.tile([128, 1], f32)
        nc.vector.memset(eps_t[:], EPS2)

        # --- big SBUF arrays ---
        twg = big.tile([128, NTWB, D], bf16)     # gathered tw rows, tight
        ug = big.tile([128, NUB, E], bf16)       # gathered c_emb rows, tight
        tpT_all = big.tile([D, NTWB * 128], bf16)  # feature-major tp
        uT_all = big.tile([E, NU], bf16)         # feature-major u
        cpT_all = big.tile([D, NU], bf16)        # context projection (+Ac_b)
        tp2_all = big.tile([D, NTWB * 128], bf16)
        cp2_all = big.tile([D, NU], bf16)
        bu_all = big.tile([M, BLOC * E], bf16)   # u @ (R Bc)^T + rbeff
        attnT = big.tile([M, BLOC * J], bf16)
        ntinv = big.tile([128, BLOC], f32)
        ncinv = big.tile([M, BLOC], f32)
        lnt_t = big.tile([128, BLOC], f32)
        lnc_t = big.tile([M, BLOC], f32)
        rs_all = big.tile([128, BLOC], f32)

        # --- gathers: 128 rows per indirect DMA (one desc per partition).
        # u blocks first (they feed the early u-side pipeline), then tw.
        for c in range(NUB):
            nc.gpsimd.indirect_dma_start(
                out=ug[:, c, :], out_offset=None, in_=cemb[:, :],
                in_offset=bass.IndirectOffsetOnAxis(
                    ap=offc_t[:, c : c + 1], axis=0
                ),
            )
        for c in range(NTWB):
            nc.gpsimd.indirect_dma_start(
                out=twg[:, c, :], out_offset=None, in_=twe[:, :],
                in_offset=bass.IndirectOffsetOnAxis(
                    ap=offt_t[:, c : c + 1], axis=0
                ),
            )

        if True:
            ps_tr = ctx.enter_context(
                tc.tile_pool(name="ps_tr", bufs=1, space="PSUM"))
            ps_sm = ctx.enter_context(
                tc.tile_pool(name="ps_sm", bufs=1, space="PSUM"))

            # nt columns and cs columns share one PSUM bank
            sm_ps = ps_sm.tile([128, 128], f32, tag="sm", space="PSUM")
            nt_ps = sm_ps[:, :BLOC]
            cs_ps = sm_ps[:, BLOC : 2 * BLOC]

            # ---- u side: transposes, projection, bu, context norms ----
            with tc.tile_pool(name="ps_pj", bufs=2, space="PSUM") as ps_pj, \
                 tc.tile_pool(name="ps_nc", bufs=1, space="PSUM") as ps_nc:
                for g in range((NUB + 7) // 8):
                    nblk = min(8, NUB - 8 * g)
                    floor_ms = (4600 + 1038 * (8 * g + nblk)) * 1e-6
                    with tc.tile_wait_until(floor_ms):
                        tr = ps_tr.tile([128, 1024], bf16, tag="tr",
                                        space="PSUM")
                        for i in range(nblk):
                            nc.tensor.transpose(
                                out=tr[:, 128 * i : 128 * (i + 1)],
                                in_=ug[:, 8 * g + i, :],
                                identity=eye_t[:, :],
                            )
                        nc.vector.tensor_copy(
                            out=uT_all[:, 1024 * g : 1024 * g + 128 * nblk],
                            in_=tr[:, : 128 * nblk],
                        )
                for c in range((NU + 511) // 512):
                    s = 512 * c
                    w = min(512, NU - s)
                    pj = ps_pj.tile([128, 512], f32, tag="pj", space="PSUM")
                    nc.tensor.matmul(
                        out=pj[:D, :w], lhsT=acw_t[:], rhs=uT_all[:, s : s + w],
                        start=True, stop=True,
                    )
                    if c % 2 == 0:
                        nc.vector.tensor_scalar_add(
                            out=cpT_all[:, s : s + w], in0=pj[:D, :w],
                            scalar1=acb_t[:],
                        )
                    else:
                        nc.scalar.activation(
                            out=cpT_all[:, s : s + w], in_=pj[:D, :w],
                            func=AF.Identity, bias=acb_t[:], scale=1.0,
                        )
                    nc.vector.tensor_mul(
                        out=cp2_all[:, s : s + w],
                        in0=cpT_all[:, s : s + w],
                        in1=cpT_all[:, s : s + w],
                    )
                nc_ps = ps_nc.tile([M, BLOC], f32, tag="ncn", space="PSUM")
                for b in range(BLOC):
                    nc.tensor.matmul(
                        out=nc_ps[:, b : b + 1],
                        lhsT=cp2_all[:, M * b : M * (b + 1)],
                        rhs=ones_t[:D, :],
                        start=True, stop=True,
                    )
                nc.scalar.activation(
                    out=lnc_t[:], in_=nc_ps[:, :], func=AF.Ln, bias=eps_t[:M]
                )
                nc.scalar.activation(
                    out=ncinv[:], in_=lnc_t[:], func=AF.Exp, scale=-0.5
                )
                for c in range(BLOC // 4):
                    bu_ps = ps_pj.tile([128, 512], f32, tag="pj", space="PSUM")
                    for i in range(4):
                        b = 4 * c + i
                        nc.tensor.matmul(
                            out=bu_ps[:M, 128 * i : 128 * (i + 1)],
                            lhsT=uT_all[:, M * b : M * b + M],
                            rhs=wrb_t[:],
                            start=True, stop=True,
                        )
                    nc.vector.scalar_tensor_tensor(
                        out=bu_all[:, 512 * c : 512 * (c + 1)],
                        in0=bu_ps[:M, :], scalar=1.0,
                        in1=rb4_t[:],
                        op0=ALU.mult, op1=ALU.add,
                    )

            # ---- tw side + attention, interleaved with the gather stream ----
            # One PSUM bank per in-flight batch: dot at byte 0 (f32 [101,50]),
            # dotT at byte 1024 (bf16 [50,128] via bitcast). 4 banks = depth 4.
            ps_ch = ctx.enter_context(
                tc.tile_pool(name="ps_ch", bufs=4, space="PSUM"))
            ps_o = ctx.enter_context(
                tc.tile_pool(name="ps_o", bufs=2, space="PSUM"))
            GRP = 16
            pendB = []
            obuf = None

            def stageA(b):
                s = J * b
                ch = ps_ch.tile([128, 512], f32, tag="ch", space="PSUM")
                dot_ps = ch[:J, :M]
                nc.tensor.matmul(
                    out=dot_ps,
                    lhsT=tpT_all[:, s : s + J],
                    rhs=cpT_all[:, M * b : M * b + M],
                    start=True, stop=True,
                )
                dotn = dotp.tile([J, M], bf16, tag="dotn")
                nc.vector.tensor_scalar_mul(
                    dotn[:], dot_ps, ntinv[:J, b : b + 1]
                )
                dT_ps = ch[:M, 256:320].bitcast(bf16)
                nc.tensor.transpose(
                    out=dT_ps[:, :J], in_=dotn[:], identity=eye_t[:J, :J]
                )
                nc.scalar.activation(
                    out=attnT[:, s : s + J],
                    in_=dT_ps[:, :J],
                    func=AF.Exp,
                    bias=maskT_t[:, b : b + 1],
                    scale=ncinv[:, b : b + 1],
                )

            def stageCS(b):
                s = J * b
                nc.tensor.matmul(
                    out=cs_ps[:J, b : b + 1],
                    lhsT=attnT[:, s : s + J],
                    rhs=ones_t[:M, :],
                    start=True, stop=True,
                )

            def stageB(b, buf):
                s = J * b
                o_ps = ps_o.tile([J, E], f32, tag="o", space="PSUM")
                nc.tensor.matmul(
                    out=o_ps[:, :],
                    lhsT=attnT[:, s : s + J],
                    rhs=bu_all[:, E * b : E * (b + 1)],
                    start=True, stop=True,
                )
                k = b % 8
                if True:
                    nc.vector.tensor_scalar_mul(
                        buf[:, E * k : E * (k + 1)], o_ps[:, :],
                        rs_all[:J, b : b + 1],
                    )
                else:
                    nc.scalar.activation(
                        out=buf[:, E * k : E * (k + 1)], in_=o_ps[:, :],
                        func=AF.Identity, scale=rs_all[:J, b : b + 1],
                    )

            def flush(b0, n, buf):
                k0 = (b0 % 8) * E
                dst = bass.AP(
                    tensor=out.tensor,
                    offset=b0 * J * E,
                    ap=[[E, J], [J * E, n], [1, E]],
                )
                nc.sync.dma_start(out=dst, in_=buf[:, k0 : k0 + n * E])

            fbase = [0]

            def runB():
                nonlocal obuf
                b = pendB.pop(0)
                if obuf is None:
                    obuf = outp.tile([J, 8 * E], f32, tag="ob")
                    fbase[0] = b
                stageB(b, obuf)
                n = b - fbase[0] + 1
                if n == 8 or b == BLOC - 1 or (
                        b >= BLOC - 8 and (b + 1) % 4 == 0) or (
                        b >= BLOC - 8 and (b + 1) % 2 == 0) or b >= BLOC - 2:
                    flush(fbase[0], n, obuf)
                    obuf = None

            def window(lo, hi):
                # ntinv for batches [lo, hi), then their softmax chain
                n = hi - lo
                nc.scalar.activation(
                    out=lnt_t[:J, lo:hi], in_=nt_ps[:J, lo:hi],
                    func=AF.Ln, bias=eps_t[:J],
                )
                nc.scalar.activation(
                    out=ntinv[:J, lo:hi], in_=lnt_t[:J, lo:hi],
                    func=AF.Exp, scale=-0.5,
                )
                for b in range(lo, hi):
                    stageA(b)
                    if pendB:
                        runB()
                for b in range(lo, hi):
                    stageCS(b)
                nc.vector.reciprocal(
                    out=rs_all[:J, lo:hi], in_=cs_ps[:J, lo:hi],
                )
                pendB.extend(range(lo, hi))

            done_b = 0   # batches whose nt-norm matmul has been emitted
            blk0 = 0
            GSIZES = [8, 8, 8, 8, 8, 4, 4, 3]
            assert sum(GSIZES) == NTWB
            for nblk in GSIZES:
                floor_ms = (4600 + 1038 * (NUB + blk0 + nblk)) * 1e-6
                with tc.tile_wait_until(floor_ms):
                    tr = ps_tr.tile([128, 1024], bf16, tag="tr", space="PSUM")
                    for i in range(nblk):
                        nc.tensor.transpose(
                            out=tr[:D, 128 * i : 128 * (i + 1)],
                            in_=twg[:, blk0 + i, :],
                            identity=eye_t[:, :],
                        )
                    s2 = 128 * blk0
                    nc.vector.tensor_copy(
                        out=tpT_all[:, s2 : s2 + 128 * nblk],
                        in_=tr[:D, : 128 * nblk],
                    )
                    nc.vector.tensor_mul(
                        out=tp2_all[:, s2 : s2 + 128 * nblk],
                        in0=tpT_all[:, s2 : s2 + 128 * nblk],
                        in1=tpT_all[:, s2 : s2 + 128 * nblk],
                    )
                    blk0 += nblk
                    nb = min(BLOC, (128 * blk0) // J)
                    for b in range(done_b, nb):
                        nc.tensor.matmul(
                            out=nt_ps[:J, b : b + 1],
                            lhsT=tp2_all[:, J * b : J * (b + 1)],
                            rhs=ones_t[:D, :],
                            start=True, stop=True,
                        )
                    if nb > done_b:
                        window(done_b, nb)
                    done_b = nb
            while pendB:
                runB()

    if NOPATCH:
        nc.compile()
    else:
        orig = bacc_mod.get_activation_tables
        bacc_mod.get_activation_tables = _patched_tables(orig)
        try:
            nc.compile()
        finally:
            bacc_mod.get_activation_tables = orig
    return nc


def _get_program():
    if "nc" not in _CACHE:
        _CACHE["nc"] = _build_program()
    return _CACHE["nc"]


def _prep_inputs(batch_titems, batch_citems, batch_pad_ids, t_emb, c_emb,
                 Ac_w, Ac_b, At_w, At_b, Bc_w, Bc_b, R_w, R_b):
    import ml_dtypes
    bf16 = ml_dtypes.bfloat16

    f = lambda x: np.ascontiguousarray(np.asarray(x, dtype=np.float32))
    t32 = np.asarray(t_emb, np.float32)
    c32 = np.asarray(c_emb, np.float32)
    At = np.asarray(At_w, np.float32)
    Ac = np.asarray(Ac_w, np.float32)
    Bc = np.asarray(Bc_w, np.float32)
    R = np.asarray(R_w, np.float32)

    # tw rows = t_emb @ At^T + At_b (bias folded into the table)
    twe = np.ascontiguousarray(
        (t32 @ At.T + np.asarray(At_b, np.float32)).astype(bf16))
    cemb_b = np.ascontiguousarray(c32.astype(bf16))

    tit = np.asarray(batch_titems).astype(np.int32)
    cit = np.asarray(batch_citems).astype(np.int32)
    pad = np.asarray(batch_pad_ids).astype(np.int64)

    mask = np.zeros((B, M), np.float32)
    mask[pad[0], pad[1]] = NEG

    acw = np.ascontiguousarray(Ac.T.astype(bf16))
    wrb = np.ascontiguousarray((R @ Bc).T.astype(bf16))
    acb = f(np.asarray(Ac_b).reshape(D, 1))
    rbeff = f(
        (np.asarray(R_b, np.float32) + R @ np.asarray(Bc_b, np.float32)
         ).reshape(1, E)
    )
    eye = np.eye(128, dtype=np.float32).astype(bf16)

    in_maps = []
    for c in range(NCORES):
        s = c * BLOC
        # tight tw offsets: item i = (b*J + j) at partition i%128, block i//128
        tflat = tit[s : s + BLOC].reshape(-1)  # [6464]
        tpad = np.zeros(NTWB * 128, np.int32)
        tpad[:NTW] = tflat
        offt = np.ascontiguousarray(tpad.reshape(NTWB, 128).T)

        cflat = cit[s : s + BLOC].reshape(-1)  # [3200]
        offc = np.ascontiguousarray(cflat.reshape(NUB, 128).T)

        maskTc = np.ascontiguousarray(mask[s : s + BLOC].T)  # [50,64]

        in_maps.append(
            {
                "twe": twe,
                "cemb": cemb_b,
                "acw": acw,
                "wrb": wrb,
                "acb": acb,
                "rbeff": rbeff,
                "eye": eye,
                "offt": offt,
                "offc": offc,
                "maskT": maskTc,
            }
        )
    return in_maps


def run_sharded(in_maps, **kwargs):
    from concourse.bass_utils import run_bass_kernel_spmd

    nc = _get_program()
    res = run_bass_kernel_spmd(nc, in_maps, core_ids=list(range(NCORES)), **kwargs)
    outs = [res.results[c]["out"] for c in range(NCORES)]
    full = np.concatenate(outs, axis=0)
    return full, res


def kernel(**inputs):
    in_maps = _prep_inputs(**inputs)
    full, _ = run_sharded(in_maps)
    return full.astype(np.float32)
